# revision 39
# baseline (speedup 1.0000x reference)
"""Gated GQA self-attention with KV cache, tensor-parallel over heads on 8
Trainium2 NeuronCores.

Reference computation (fp32):
    q = rms_norm((x @ w_q.T).reshape(B,L,H,HD))      # per-head rms over HD
    k = rms_norm((x @ w_k.T).reshape(B,L,HKV,HD))
    v = (x @ w_v.T).reshape(B,L,HKV,HD)
    k_t/v_t = concat(cache, new) over seq -> [B,HKV,S,HD]
    o = softmax(q @ k_t.T / sqrt(HD)) @ v_t          # full (non-causal)
    o *= sigmoid(x[..., :16] @ w_gate.T)             # per-head gate
    y = o.reshape(B,L,D) @ w_out.T

Sharding: core c owns q heads {2c, 2c+1} and kv group g=c//2.  Each core
computes its heads' attention plus the partial out-projection
y_c = o_c @ w_out[:, cols_c].T; the host sums the 8 partials.

v13 design (349 -> 292 -> 253 -> 240us):
  * bf16 data plane everywhere (x / wqkv / wo / caches / q / k / v /
    exp(p) / attention out); fp32 only inside PSUM accumulation.
  * all matmuls use 512-wide moving operands: steady-state PE cadence is
    215ns per matmul (2.4 GHz, LDWEIGHTS hidden under the previous
    matmul's shadow-buffer load).
  * softmax denominator: DVE running bf16 sum of the exp tiles plus one
    tiny st128x1 reduce matmul per l-half at block end (v5 spent ~59us
    of PE time on full-width denominator matmuls).
  * gates: sigmoid evaluated via the Exp table (1/(1+e^-z), DVE finish)
    so no Sigmoid table load interrupts the attention exp stream.
  * DMA: each hwdge queue sustains only ~125 GB/s, and dependency
    tracking is tile-granular (a consumer waits for every DMA into its
    tile).  So x streams in per-DMA granule tiles - chunk 0 as four
    kq-aligned quarters on sync while the wqkv quarters ride the scalar
    queue, later chunks as one half per queue - and the tail y rows
    drain as per-queue halves.
  * projection chunks 2-3, the batch-1 rms factors, and the batch-0
    out-projection interleave into attention as PE filler units, with
    consumption capped so no block runs filler-dry and a few units stay
    in reserve to hide the last finisher's factor-chain latency.
  * tiny row<->column factor transposes ride the PE via identity
    transposes + shape-preserving DMAs; the gate/denominator factors are
    folded into the raw attention output before the out-projection.
"""

from contextlib import ExitStack

import numpy as np
import ml_dtypes

import concourse.bass as bass
import concourse.tile as tile
from concourse import bacc, mybir
from concourse.bass_utils import run_bass_kernel_spmd

F32R = mybir.dt.float32r
F32 = mybir.dt.float32
BF16 = mybir.dt.bfloat16
AF = mybir.ActivationFunctionType
AX = mybir.AxisListType
OP = mybir.AluOpType
BF = ml_dtypes.bfloat16

B, L, D = 2, 1024, 2048
H, HKV, HD = 16, 4, 128
CACHE = 1024
BL = B * L                  # 2048
S = CACHE + L               # 2048
NCORES = 8
QH = H // NCORES            # 2 q heads per core
JC = QH * HD                # 256 out-proj contraction cols per core
EPS = 1e-6
ND = D // 128               # 16 contraction chunks
NSC = CACHE // 128          # 8 cached s chunks per batch
NS = S // 128               # 16 s chunks per batch

_CACHED_NC = None


def _build_core_program():
    nc = bacc.Bacc("TRN2", target_bir_lowering=False, debug=False)

    xt4 = nc.dram_tensor("xt4", [4, 128, ND, 512], BF16, kind="ExternalInput").ap()
    wqkv = nc.dram_tensor("wqkv", [4, 128, 4, 512], BF16, kind="ExternalInput").ap()
    wo = nc.dram_tensor("wo", [128, QH, D], BF16, kind="ExternalInput").ap()
    wg = nc.dram_tensor("wg", [H, QH], BF16, kind="ExternalInput").ap()
    xg = nc.dram_tensor("xg", [H, BL], BF16, kind="ExternalInput").ap()
    ckt = nc.dram_tensor("ckt", [B, 128, CACHE], BF16, kind="ExternalInput").ap()
    cv = nc.dram_tensor("cv", [B, 128, NSC, HD], BF16, kind="ExternalInput").ap()
    identr_in = nc.dram_tensor("identr", [128, 128], F32R, kind="ExternalInput").ap()
    ones_in = nc.dram_tensor("ones_in", [128, 128], F32R, kind="ExternalInput").ap()
    y = nc.dram_tensor("y", [BL, D], BF16, kind="ExternalOutput").ap()

    with tile.TileContext(nc) as tc, ExitStack() as ctx:
        singles = ctx.enter_context(tc.tile_pool(name="singles", bufs=1))
        xtp = ctx.enter_context(tc.tile_pool(name="xtp", bufs=2))
        exp_p = ctx.enter_context(tc.tile_pool(name="exp_p", bufs=10))
        accp = ctx.enter_context(tc.tile_pool(name="accp", bufs=4))
        wkp = ctx.enter_context(tc.tile_pool(name="wkp", bufs=4))
        colp = ctx.enter_context(tc.tile_pool(name="colp", bufs=2))
        dnrp = ctx.enter_context(tc.tile_pool(name="dnrp", bufs=2))
        ysbp = ctx.enter_context(tc.tile_pool(name="ysbp", bufs=3))

        # PSUM (8 banks): psS = 5 x [128,512] transient ring (scores,
        # projection accumulators, transposes, broadcasts, phase-3
        # accumulators), psA = 3 x [128,512] persistent-per-block (p@v
        # accumulators x2 + both denominator rows stacked at partitions
        # 0/32 of one bank via matmul tile_position).
        psS = ctx.enter_context(tc.tile_pool(name="psS", bufs=5, space="PSUM"))
        psA = ctx.enter_context(tc.tile_pool(name="psA", bufs=3, space="PSUM"))

        lowp = nc.allow_low_precision(reason="bf16/f32r data plane is intended")
        ctx.enter_context(lowp)

        identr = singles.tile([128, 128], F32R)
        nc.scalar.dma_start(out=identr, in_=identr_in)
        ones128 = singles.tile([128, 128], F32R)
        nc.scalar.dma_start(out=ones128, in_=ones_in)
        onesr = ones128[0:1, :]
        identb = singles.tile([128, 128], BF16)
        nc.vector.tensor_copy(identb, identr)
        bias_q = singles.tile([128, 1], F32)
        nc.vector.memset(bias_q, HD * EPS)
        bias_k = singles.tile([128, 1], F32)
        nc.vector.memset(bias_k, EPS)

        wqkv_q = [singles.tile([128, 4, 512], BF16, tag=f"wq{k}",
                               name=f"wq{k}") for k in range(4)]
        nc.scalar.dma_start(out=wqkv_q[0], in_=wqkv[0])
        wo_sb = singles.tile([128, QH, D], BF16)
        wg_sb = singles.tile([H, QH], BF16)
        xg_sb = singles.tile([H, BL], BF16)

        # persistent activations, feature-on-partition
        qkvt = singles.tile([128, 3, BL], BF16)       # q0, q1, k (normalized)
        vstage = singles.tile([128, BL], BF16)        # raw v, [d, l]
        otg = singles.tile([128, B, QH, 2, 512], BF16)  # raw attn out [d, l]
        gcol = singles.tile([128, 16, QH], F32)       # gates, l-on-partition
        # rms rows packed at partitions 0/32/64 of one tile (one 8KB SBUF
        # column instead of three), which are also legal matmul bases
        sqr3 = singles.tile([65, BL], F32R)
        rr3 = singles.tile([65, BL], F32R)
        f2 = {b: singles.tile([33, 1024], F32R, tag=f"fr{b}", name=f"fr{b}")
              for b in range(B)}

        cache_tiles = {}
        vnew = {}

        def emit_prefetch():
            for kq in range(1, 4):
                nc.scalar.dma_start(out=wqkv_q[kq], in_=wqkv[kq])
            nc.scalar.dma_start(out=xg_sb, in_=xg)
            nc.scalar.dma_start(out=wg_sb, in_=wg)

        def emit_caches0():
            # batch-0 caches ride the scalar queue after the x1 half so
            # chunk 1 is never cache-blocked; they are only needed when
            # attention starts
            for b in range(B):
                cache_tiles.setdefault(b, [None, None])
            t = singles.tile([128, CACHE], BF16, tag="ck0", name="ck0")
            nc.scalar.dma_start(out=t, in_=ckt[0])
            cache_tiles[0][0] = t
            t = singles.tile([128, NSC, HD], BF16, tag="cv0", name="cv0")
            nc.scalar.dma_start(out=t, in_=cv[0])
            cache_tiles[0][1] = t

        def emit_prefetch_late():
            t = singles.tile([128, CACHE], BF16, tag="ck1", name="ck1")
            nc.scalar.dma_start(out=t, in_=ckt[1])
            cache_tiles[1][0] = t
            t = singles.tile([128, NSC, HD], BF16, tag="cv1", name="cv1")
            nc.scalar.dma_start(out=t, in_=cv[1])
            cache_tiles[1][1] = t
            nc.scalar.dma_start(out=wo_sb, in_=wo)

        # ---- phase 1: projections -------------------------------------
        def proj_units(cs):
            """Generator: two units per (chunk, jc) = 2x8 matmuls + evac."""
            for c in cs:
                yield from _proj_chunk(c)

        def _proj_chunk(c):
            sl = slice(c * 512, c * 512 + 512)
            if c == 0:
                # x chunk 0 streams in per-DMA quarter tiles on sync while
                # wqkv quarters ride the scalar queue: tile-granular dep
                # tracking means a shared tile would make the first matmul
                # wait for every quarter
                xq = [xtp.tile([128, 4, 512], BF16, tag=f"xq{q}",
                               name=f"xq{q}") for q in range(4)]
                nc.sync.dma_start(out=xq[0], in_=xt4[0, :, 0:4, :])
                emit_prefetch()
                for q in range(1, 4):
                    nc.sync.dma_start(
                        out=xq[q], in_=xt4[0, :, 4 * q : 4 * q + 4, :]
                    )
            else:
                # chunks 1-3 in half tiles, one per hwdge queue
                xh = [xtp.tile([128, 8, 512], BF16, tag=f"xh{hq}",
                               name=f"xh{c}_{hq}") for hq in range(2)]
                nc.sync.dma_start(out=xh[0], in_=xt4[c, :, 0:8, :])
                nc.scalar.dma_start(out=xh[1], in_=xt4[c, :, 8:16, :])
            pps = {}
            if c == 0:
                # first chunk: iterate weight quarters outermost so matmuls
                # start as soon as the first wqkv quarter + x quarter land
                for jc in (3, 2, 0, 1):
                    pps[jc] = psS.tile([128, 512], F32, tag="ps",
                                       name=f"pp{c}_{jc}")
                for kq in range(4):
                    for jc in (3, 2, 0, 1):
                        for kk in range(kq * 4, kq * 4 + 4):
                            nc.tensor.matmul(
                                pps[jc],
                                wqkv_q[kk // 4][:, kk % 4,
                                                jc * 128 : jc * 128 + 128],
                                xq[kk // 4][:, kk % 4, :],
                                start=(kk == 0),
                                stop=(kk == ND - 1),
                            )
            for jc in (3, 2, 0, 1):  # v and k first
                if c == 0:
                    pp = pps[jc]
                else:
                    pp = psS.tile([128, 512], F32, tag="ps", name=f"pp{c}_{jc}")
                    for kk in range(ND):
                        nc.tensor.matmul(
                            pp,
                            wqkv_q[kk // 4][:, kk % 4,
                                            jc * 128 : jc * 128 + 128],
                            xh[kk // 8][:, kk % 8, :],
                            start=(kk == 0),
                            stop=(kk == ND - 1),
                        )
                        if kk == 7:
                            yield  # mid-chain filler granularity
                if jc == 3:
                    nc.vector.tensor_copy(vstage[:, sl], pp)
                else:
                    r = jc if jc < 2 else 2
                    nc.vector.tensor_copy(qkvt[:, r, sl], pp)
                    sq = wkp.tile([128, 512], BF16, tag="sq", name=f"sq{c}_{jc}")
                    nc.vector.tensor_mul(sq, qkvt[:, r, sl], qkvt[:, r, sl])
                    ssq = psS.tile([128, 512], F32, tag="ps", name=f"ssq{c}_{jc}")
                    p = 32 * r
                    nc.tensor.matmul(
                        ssq[p : p + 1, :], onesb, sq, start=True, stop=True
                    )
                    # raw sum-of-squares; sqrt happens 128-lane in column
                    # form inside finish_half so projection units emit no
                    # ACT work (keeps the Exp table loaded during
                    # interleaved attention)
                    nc.vector.tensor_copy(
                        sqr3[p : p + 1, sl], ssq[p : p + 1, :]
                    )
                yield

        def finish_half(half):
            """Reciprocal of the three rms rows (q0, q1, k): split each row
            to 8 partitions with a shape-preserving SBUF-SBUF DMA, PE-
            transpose into columns, 128-lane DVE reciprocal, transpose
            back, re-flatten; then rank-1 normalize of qkvt columns."""
            row_sl = slice(half * 1024, half * 1024 + 1024)
            st8 = colp.tile([8, 3, 128], F32R, tag="st8", name=f"st8_{half}")
            for r in range(3):
                nc.gpsimd.dma_start(
                    out=st8[:, r, :], in_=sqr3[32 * r : 32 * r + 1, row_sl]
                )
            tpc = psS.tile([128, 24], F32R, tag="ps", name=f"tpc{half}")
            for r in range(3):
                nc.tensor.transpose(
                    tpc[:, r * 8 : r * 8 + 8], st8[:, r, :], identr[0:8, 0:8]
                )
            # cols hold raw ssq: rsqrt = reciprocal(sqrt(ssq*scale + bias))
            # (q rows fold the 1/sqrt(HD) score scale into the bias form)
            sq_c = colp.tile([128, 24], F32R, tag="sqc", name=f"sqc{half}")
            for r in range(3):
                scale = 1.0 if r < 2 else 1.0 / HD
                bias = bias_q if r < 2 else bias_k
                nc.scalar.activation(
                    sq_c[:, r * 8 : r * 8 + 8], tpc[:, r * 8 : r * 8 + 8],
                    AF.Sqrt, bias=bias[:, :], scale=scale,
                )
            colsb = colp.tile([128, 24], F32R, tag="rcol", name=f"rcol{half}")
            nc.vector.reciprocal(colsb, sq_c)
            tpr = psS.tile([8, 384], F32R, tag="ps", name=f"tpr{half}")
            for r in range(3):
                nc.tensor.transpose(
                    tpr[:, r * 128 : r * 128 + 128],
                    colsb[:, r * 8 : r * 8 + 8],
                    identr,
                )
            st8b = colp.tile([8, 3, 128], F32R, tag="st8b", name=f"st8b{half}")
            for r in range(3):
                nc.vector.tensor_copy(
                    st8b[:, r, :], tpr[:, r * 128 : r * 128 + 128]
                )
            for r in range(3):
                nc.gpsimd.dma_start(
                    out=rr3[32 * r : 32 * r + 1, row_sl], in_=st8b[:, r, :]
                )
            for r in range(3):
                p = 32 * r
                for lc in range(2):
                    sl = slice(half * 1024 + lc * 512, half * 1024 + lc * 512 + 512)
                    bc = psS.tile([128, 512], F32, tag="ps",
                                  name=f"nbc{half}_{r}_{lc}")
                    nc.tensor.matmul(
                        bc, ones128[p : p + 1, :], rr3[p : p + 1, sl],
                        start=True, stop=True,
                    )
                    nc.vector.tensor_mul(qkvt[:, r, sl], qkvt[:, r, sl], bc)

        def emit_gates():
            # gates in column form: [l-part, chunk, head]
            gps = psS.tile([128, 16, QH], F32, tag="ps", name="gps")
            for cc in range(16):
                nc.tensor.matmul(
                    gps[:, cc, :],
                    xg_sb[:, cc * 128 : cc * 128 + 128],
                    wg_sb,
                    start=True,
                    stop=True,
                )
            nc.scalar.activation(gcol, gps, AF.Exp, scale=-1.0)
            nc.vector.tensor_scalar_add(gcol, gcol, 1.0)
            nc.vector.reciprocal(gcol, gcol)

        # ---- phase 2+3: attention and out-projection ------------------
        def prep_v(b, i0=0, i1=NSC):
            """Transpose new v [d,l] -> [s,d] bf16 for this batch."""
            if b not in vnew:
                vnew[b] = singles.tile([128, NSC, HD], BF16, tag=f"vn{b}",
                                       name=f"vn{b}")
            vn = vnew[b]
            for i in range(i0, i1):
                tp = psS.tile([128, 128], BF16, tag="ps", name=f"tp{b}_{i}")
                nc.tensor.transpose(
                    tp,
                    vstage[:, b * L + i * 128 : b * L + i * 128 + 128],
                    identb,
                )
                nc.vector.tensor_copy(vn[:, i, :], tp)

        def attn(b, h, filler=None, pending=None):
            """Attention for (batch b, local head h), pipelined 2 s-chunks
            ahead.  filler() emits one unit of foreign PE work per s-chunk
            from sc 5 on; pending is the previous block's deferred factor
            chain, emitted at s-chunk 4.  The softmax denominator is
            accumulated on DVE (bf16 running sum of the exp tiles) and
            partition-reduced with a single tiny matmul per l-half at the
            end, instead of 32 full-width st128x1 PE matmuls."""
            boff = b * L
            ck_sb, cv_sb = cache_tiles[b]
            ot = [psA.tile([128, 512], F32, tag="pa", name=f"ot{b}{h}{i}")
                  for i in range(2)]
            den = psA.tile([33, 512], F32, tag="pa", name=f"dn{b}{h}")
            acc = [accp.tile([128, 512], BF16, tag=f"ac{lh}",
                             name=f"ac{b}{h}{lh}") for lh in range(2)]
            exs = {}
            vxs = {}
            for sc in range(NS + 4):
                if sc < NS:
                    if sc < NSC:
                        kT = ck_sb[:, sc * 128 : sc * 128 + 128]
                        vxs[sc] = cv_sb[:, sc, :]
                    else:
                        j = boff + (sc - NSC) * 128
                        kT = qkvt[:, 2, j : j + 128]
                        vxs[sc] = vnew[b][:, sc - NSC, :]
                    for lh in range(2):
                        sp = psS.tile([128, 512], F32, tag="ps",
                                      name=f"sp{b}_{h}_{sc}_{lh}")
                        nc.tensor.matmul(
                            sp, kT,
                            qkvt[:, h, boff + lh * 512 : boff + lh * 512 + 512],
                            start=True, stop=True,
                        )
                        ex = exp_p.tile([128, 512], BF16, tag="ex",
                                        name=f"ex{b}_{h}_{sc}_{lh}")
                        nc.scalar.activation(ex, sp, AF.Exp)
                        exs[(sc, lh)] = ex
                        if sc == 1:
                            nc.vector.tensor_add(
                                acc[lh], exs[(0, lh)], ex
                            )
                        elif sc > 1:
                            nc.vector.tensor_add(acc[lh], acc[lh], ex)
                if sc >= 4:
                    sc2 = sc - 4
                    vx = vxs.pop(sc2)
                    for lh in range(2):
                        ex2 = exs.pop((sc2, lh))
                        nc.tensor.matmul(
                            ot[lh], vx, ex2,
                            start=(sc2 == 0), stop=(sc2 == NS - 1),
                        )
                    if sc == 5 and pending is not None:
                        pending()
                    if filler is not None and sc >= (6 if pending else 2):
                        filler()
            for lh in range(2):
                nc.tensor.matmul(
                    den[32 * lh : 32 * lh + 1, :], onesb, acc[lh],
                    start=True, stop=True,
                )
            # raw-evacuate attention out so the PSUM accumulators recycle
            for lh in range(2):
                nc.vector.tensor_copy(otg[:, b, h, lh, :], ot[lh])
            dnr = dnrp.tile([33, 512], F32R, tag="dnr", name=f"dnr{b}{h}")
            # ACT evacuates the den rows (it is idle at block boundaries
            # while DVE drains the otg copies), so the psA bank frees for
            # the next block's accumulators sooner
            for lh in range(2):
                nc.scalar.copy(
                    dnr[32 * lh : 32 * lh + 1, :],
                    den[32 * lh : 32 * lh + 1, :],
                )
            st4 = colp.tile([4, 2, 128], F32R, tag="st4", name=f"st4{b}{h}")
            for lh in range(2):
                nc.gpsimd.dma_start(
                    out=st4[:, lh, :], in_=dnr[32 * lh : 32 * lh + 1, :]
                )

            def finisher():
                # den rows -> columns (PE transpose), reciprocal, combine
                # with gates, back to rows, broadcast onto otg
                tpd = psS.tile([128, 8], F32R, tag="ps", name=f"tpd{b}{h}")
                for lh in range(2):
                    nc.tensor.transpose(
                        tpd[:, lh * 4 : lh * 4 + 4], st4[:, lh, :],
                        identr[0:4, 0:4],
                    )
                dcol = colp.tile([128, 8], F32R, tag="dcol", name=f"dc{b}{h}")
                nc.vector.reciprocal(dcol, tpd)
                fcol = colp.tile([128, 8], F32R, tag="fcol", name=f"fc{b}{h}")
                nc.vector.tensor_mul(
                    fcol, dcol, gcol[:, 8 * b : 8 * b + 8, h]
                )
                tpf = psS.tile([4, 256], F32R, tag="ps", name=f"tpf{b}{h}")
                st4b = colp.tile([4, 2, 128], F32R, tag="st4b",
                                 name=f"st4b{b}{h}")
                for lh in range(2):
                    nc.tensor.transpose(
                        tpf[:, lh * 128 : lh * 128 + 128],
                        fcol[:, lh * 4 : lh * 4 + 4],
                        identr,
                    )
                    nc.vector.tensor_copy(
                        st4b[:, lh, :], tpf[:, lh * 128 : lh * 128 + 128]
                    )
                    nc.gpsimd.dma_start(
                        out=f2[b][32 * h : 32 * h + 1,
                                  lh * 512 : lh * 512 + 512],
                        in_=st4b[:, lh, :],
                    )
                for lh in range(2):
                    bc = psS.tile([128, 512], F32, tag="ps",
                                  name=f"fbc{b}_{h}_{lh}")
                    nc.tensor.matmul(
                        bc, ones128[32 * h : 32 * h + 1, :],
                        f2[b][32 * h : 32 * h + 1, lh * 512 : lh * 512 + 512],
                        start=True, stop=True,
                    )
                    nc.vector.tensor_mul(
                        otg[:, b, h, lh, :], otg[:, b, h, lh, :], bc
                    )
            return finisher

        def ph3_units(b):
            """Generator of phase-3 units for batch b: each unit is one
            (lc2, li, mc-pair) -> two double-head [128,512] matmuls +
            evacuation, or a row DMA."""
            n = 0
            for lh in range(2):
                for li in range(4):
                    ysb = ysbp.tile([128, D], BF16, tag="ysb",
                                    name=f"ysb{b}_{lh}_{li}")
                    for mcp in range(2):
                        yps = []
                        for i in range(2):
                            mc = mcp * 2 + i
                            yp = psS.tile([128, 512], F32, tag="ps",
                                          name=f"yp{b}_{lh}_{li}_{mc}")
                            for hh in range(QH):
                                nc.tensor.matmul(
                                    yp,
                                    otg[:, b, hh, lh, li * 128 : li * 128 + 128],
                                    wo_sb[:, hh, mc * 512 : mc * 512 + 512],
                                    start=(hh == 0),
                                    stop=(hh == QH - 1),
                                )
                            yps.append((mc, yp))
                        for mc, yp in yps:
                            n += 1
                            if b == 1 and n % 2 == 0:
                                nc.scalar.copy(
                                    ysb[:, mc * 512 : mc * 512 + 512], yp
                                )
                            else:
                                nc.vector.tensor_copy(
                                    ysb[:, mc * 512 : mc * 512 + 512], yp
                                )
                        yield
                    row0 = b * L + lh * 512 + li * 128
                    if b == 0:
                        nc.sync.dma_start(out=y[row0 : row0 + 128, :], in_=ysb)
                    else:
                        # tail rows drain as per-queue halves so the final
                        # writes finish ~4us sooner
                        nc.sync.dma_start(
                            out=y[row0 : row0 + 128, 0:1024], in_=ysb[:, 0:1024]
                        )
                        nc.scalar.dma_start(
                            out=y[row0 : row0 + 128, 1024:2048],
                            in_=ysb[:, 1024:2048],
                        )
                    yield

        def drain(gen):
            if gen is not None:
                for _ in gen:
                    pass

        # ---- emission sequence ----------------------------------------
        onesb = singles.tile([128, 1], BF16)
        nc.vector.tensor_copy(onesb, ones128[:, 0:1])
        drain(proj_units(range(0, 1)))
        # gates and the first four v transposes need only chunk-0 data:
        # they fill the PE while the chunk-1 x tile streams in
        emit_gates()
        prep_v(0, 0, 4)
        drain(proj_units(range(1, 2)))
        emit_caches0()
        emit_prefetch_late()
        def late_units():
            finish_half(1)
            yield
            prep_v(1)
            yield

        import itertools
        gp = itertools.chain(proj_units(range(2, 4)), late_units())
        # remaining v transposes + three chunk-4 projection units first:
        # their PE work hides finish_half's bounce-chain latency
        prep_v(0, 4, NSC)
        finish_half(0)
        # split the remaining projection units between the two batch-0
        # attention blocks so neither runs filler-dry
        gp_left = {"n": 10}
        def gp_limited():
            if gp_left["n"] > 0:
                gp_left["n"] -= 1
                return next(gp, None)
        fin = attn(0, 0, filler=gp_limited)
        fin = attn(0, 1, filler=lambda: next(gp, None), pending=fin)
        drain(gp)
        g0 = ph3_units(0)
        fin = attn(1, 0, filler=lambda: next(g0, None), pending=fin)
        # reserve a few phase-3 units so the last finisher's factor-chain
        # latency is hidden under leftover PE work
        g0_left = {"n": 6}
        def g0_limited():
            if g0_left["n"] > 0:
                g0_left["n"] -= 1
                return next(g0, None)
        fin = attn(1, 1, filler=g0_limited, pending=fin)
        fin()
        drain(g0)
        drain(ph3_units(1))

    nc.compile()
    return nc


def _get_nc():
    global _CACHED_NC
    if _CACHED_NC is None:
        _CACHED_NC = _build_core_program()
    return _CACHED_NC


def make_in_maps(x, w_q, w_k, w_v, w_out, w_gate, cache_k, cache_v):
    xt = np.ascontiguousarray(x.reshape(BL, D).T)         # [D, BL] f32
    # [4, 128, 16, 512]: chunk, partition, k-chunk, col
    xt4 = np.ascontiguousarray(
        xt.reshape(ND, 128, 4, 512).transpose(2, 1, 0, 3)
    ).astype(BF)
    xg = xt[0:H, :].astype(BF)
    identr = np.eye(128, dtype=np.float32)
    ones128_np = np.ones((128, 128), dtype=np.float32)
    in_maps = []
    for c in range(NCORES):
        g = c // 2
        wq_c = w_q[c * JC : (c + 1) * JC]                      # [256, D]
        wk_c = w_k[g * HD : (g + 1) * HD]                      # [128, D]
        wv_c = w_v[g * HD : (g + 1) * HD]
        wqkv_c = np.concatenate([wq_c, wk_c, wv_c], axis=0).T  # [D, 512]
        wqkv4 = np.ascontiguousarray(
            wqkv_c.reshape(4, 4, 128, 512).transpose(0, 2, 1, 3)
        ).astype(BF)                                           # [4,128,4,512]
        wo_c = np.ascontiguousarray(
            w_out[:, c * JC : (c + 1) * JC].T.reshape(QH, 128, D).transpose(1, 0, 2)
        ).astype(BF)                                           # [128, 2, D] bf16
        wg_c = np.ascontiguousarray(w_gate[c * QH : (c + 1) * QH].T).astype(BF)
        ckt_c = np.ascontiguousarray(
            cache_k[:, g].transpose(0, 2, 1)
        ).astype(BF)                                           # [B, HD, CACHE]
        cv_c = np.ascontiguousarray(
            cache_v[:, g].reshape(B, NSC, 128, HD).transpose(0, 2, 1, 3)
        ).astype(BF)                                           # [B,128,NSC,HD]
        in_maps.append(
            {
                "xt4": xt4,
                "wqkv": wqkv4,
                "wo": wo_c,
                "wg": wg_c,
                "xg": xg,
                "ckt": ckt_c,
                "cv": cv_c,
                "identr": identr,
                "ones_in": ones128_np,
            }
        )
    return in_maps


def kernel(x, w_q, w_k, w_v, w_out, w_gate, cache_k, cache_v, _run_kwargs=None):
    in_maps = make_in_maps(x, w_q, w_k, w_v, w_out, w_gate, cache_k, cache_v)
    nc = _get_nc()
    res = run_bass_kernel_spmd(
        nc, in_maps, core_ids=list(range(NCORES)), **(_run_kwargs or {})
    )
    acc = np.zeros((BL, D), dtype=np.float64)
    for c in range(NCORES):
        acc += np.asarray(res.results[c]["y"], dtype=np.float32)
    out = acc.astype(np.float32).reshape(B, L, D)
    if _run_kwargs:
        kernel.last_results = res
    return out



# revision 40
# speedup vs baseline: 1.0372x; 1.0372x over previous
"""Gated GQA self-attention with KV cache, tensor-parallel over heads on 8
Trainium2 NeuronCores.

Reference computation (fp32):
    q = rms_norm((x @ w_q.T).reshape(B,L,H,HD))      # per-head rms over HD
    k = rms_norm((x @ w_k.T).reshape(B,L,HKV,HD))
    v = (x @ w_v.T).reshape(B,L,HKV,HD)
    k_t/v_t = concat(cache, new) over seq -> [B,HKV,S,HD]
    o = softmax(q @ k_t.T / sqrt(HD)) @ v_t          # full (non-causal)
    o *= sigmoid(x[..., :16] @ w_gate.T)             # per-head gate
    y = o.reshape(B,L,D) @ w_out.T

Sharding: core c owns q heads {2c, 2c+1} and kv group g=c//2.  Each core
computes its heads' attention plus the partial out-projection
y_c = o_c @ w_out[:, cols_c].T; the host sums the 8 partials.

v5 design (baseline was 349us):
  * bf16 data plane for x / wqkv / caches / q / k / exp(p); float32r
    out-projection.  Host pre-chunks every tensor so each DMA moves
    contiguous 8KB-per-partition rows.
  * everything is computed feature-on-partition; scores come out [s, l]
    so the softmax matrix feeds the p@v matmul untransposed.
  * softmax denominator is computed OFF the tensor engine: exp tiles are
    pair-summed on DVE (bf16) and cross-partition-reduced on the idle
    GPSIMD engine (axis=C), saving 65k PE rows and two PSUM banks.
  * exp runs 1024 wide (one ACT op per s-chunk) with the score pipeline
    emitted two chunks ahead, so neither ACT latency nor per-op overhead
    stalls the PE.
  * tiny row<->column factor transposes ride the PE via identity
    transposes + shape-preserving DMAs (a transposing DMA emits 4-byte
    descriptors and takes ~15us); all of them are deferred off the PE
    critical path and issued on the otherwise idle Pool queue.
  * gate/denominator factors are folded into the raw attention output
    before the out-projection, so phase 3 accumulates both heads in PSUM
    and evacuates with one copy; phase 3 of batch 0 interleaves into
    batch 1's attention emission to fill PE stall slots.
"""

from contextlib import ExitStack

import numpy as np
import ml_dtypes

import concourse.bass as bass
import concourse.tile as tile
from concourse import bacc, mybir
from concourse.bass_utils import run_bass_kernel_spmd

F32R = mybir.dt.float32r
F32 = mybir.dt.float32
BF16 = mybir.dt.bfloat16
AF = mybir.ActivationFunctionType
AX = mybir.AxisListType
OP = mybir.AluOpType
BF = ml_dtypes.bfloat16

B, L, D = 2, 1024, 2048
H, HKV, HD = 16, 4, 128
CACHE = 1024
BL = B * L                  # 2048
S = CACHE + L               # 2048
NCORES = 8
QH = H // NCORES            # 2 q heads per core
JC = QH * HD                # 256 out-proj contraction cols per core
EPS = 1e-6
ND = D // 128               # 16 contraction chunks
NSC = CACHE // 128          # 8 cached s chunks per batch
NS = S // 128               # 16 s chunks per batch

_CACHED_NC = None


def _build_core_program():
    nc = bacc.Bacc("TRN2", target_bir_lowering=False, debug=False)

    xt4 = nc.dram_tensor("xt4", [4, 128, ND, 512], BF16, kind="ExternalInput").ap()
    wqkv = nc.dram_tensor("wqkv", [4, 128, 4, 512], BF16, kind="ExternalInput").ap()
    wo = nc.dram_tensor("wo", [128, QH, D], BF16, kind="ExternalInput").ap()
    wg = nc.dram_tensor("wg", [H, QH], BF16, kind="ExternalInput").ap()
    xg = nc.dram_tensor("xg", [H, BL], BF16, kind="ExternalInput").ap()
    ckt = nc.dram_tensor("ckt", [B, 128, CACHE], BF16, kind="ExternalInput").ap()
    cv = nc.dram_tensor("cv", [B, 128, NSC, HD], BF16, kind="ExternalInput").ap()
    identr_in = nc.dram_tensor("identr", [128, 128], F32R, kind="ExternalInput").ap()
    ones_in = nc.dram_tensor("ones_in", [128, 128], F32R, kind="ExternalInput").ap()
    y = nc.dram_tensor("y", [BL, D], BF16, kind="ExternalOutput").ap()

    with tile.TileContext(nc) as tc, ExitStack() as ctx:
        singles = ctx.enter_context(tc.tile_pool(name="singles", bufs=1))
        xtp = ctx.enter_context(tc.tile_pool(name="xtp", bufs=2))
        exp_p = ctx.enter_context(tc.tile_pool(name="exp_p", bufs=10))
        accp = ctx.enter_context(tc.tile_pool(name="accp", bufs=4))
        wkp = ctx.enter_context(tc.tile_pool(name="wkp", bufs=4))
        colp = ctx.enter_context(tc.tile_pool(name="colp", bufs=2))
        dnrp = ctx.enter_context(tc.tile_pool(name="dnrp", bufs=2))
        ysbp = ctx.enter_context(tc.tile_pool(name="ysbp", bufs=3))

        # PSUM (8 banks): psS = 5 x [128,512] transient ring (scores,
        # projection accumulators, transposes, broadcasts, phase-3
        # accumulators), psA = 3 x [128,512] persistent-per-block (p@v
        # accumulators x2 + both denominator rows stacked at partitions
        # 0/32 of one bank via matmul tile_position).
        psS = ctx.enter_context(tc.tile_pool(name="psS", bufs=5, space="PSUM"))
        psA = ctx.enter_context(tc.tile_pool(name="psA", bufs=3, space="PSUM"))

        lowp = nc.allow_low_precision(reason="bf16/f32r data plane is intended")
        ctx.enter_context(lowp)

        identr = singles.tile([128, 128], F32R)
        nc.scalar.dma_start(out=identr, in_=identr_in)
        ones128 = singles.tile([128, 128], F32R)
        nc.scalar.dma_start(out=ones128, in_=ones_in)
        onesr = ones128[0:1, :]
        identb = singles.tile([128, 128], BF16)
        nc.vector.tensor_copy(identb, identr)
        bias_q = singles.tile([128, 1], F32)
        nc.vector.memset(bias_q, HD * EPS)
        bias_k = singles.tile([128, 1], F32)
        nc.vector.memset(bias_k, EPS)

        wqkv_q = [singles.tile([128, 4, 512], BF16, tag=f"wq{k}",
                               name=f"wq{k}") for k in range(4)]
        nc.scalar.dma_start(out=wqkv_q[0], in_=wqkv[0])
        wo_sb = singles.tile([128, QH, D], BF16)
        wg_sb = singles.tile([H, QH], BF16)
        xg_sb = singles.tile([H, BL], BF16)

        # persistent activations, feature-on-partition
        qkvt = singles.tile([128, 3, BL], BF16)       # q0, q1, k (normalized)
        vstage = singles.tile([128, BL], BF16)        # raw v, [d, l]
        otg = singles.tile([128, B, QH, 2, 512], BF16)  # raw attn out [d, l]
        gcol = singles.tile([128, 16, QH], F32)       # gates, l-on-partition
        # rms rows packed at partitions 0/32/64 of one tile (one 8KB SBUF
        # column instead of three), which are also legal matmul bases
        sqr3 = singles.tile([65, BL], F32R)
        rr3 = singles.tile([65, BL], F32R)
        f2 = {b: singles.tile([33, 1024], F32R, tag=f"fr{b}", name=f"fr{b}")
              for b in range(B)}

        cache_tiles = {}
        vnew = {}

        def emit_prefetch():
            for kq in range(1, 4):
                nc.scalar.dma_start(out=wqkv_q[kq], in_=wqkv[kq])
            nc.scalar.dma_start(out=xg_sb, in_=xg)
            nc.scalar.dma_start(out=wg_sb, in_=wg)

        def emit_caches0():
            # batch-0 caches ride the scalar queue after the x1 half so
            # chunk 1 is never cache-blocked
            for b in range(B):
                cache_tiles.setdefault(b, [None, None])
            t = singles.tile([128, CACHE], BF16, tag="ck0", name="ck0")
            nc.scalar.dma_start(out=t, in_=ckt[0])
            cache_tiles[0][0] = t
            t = singles.tile([128, NSC, HD], BF16, tag="cv0", name="cv0")
            nc.scalar.dma_start(out=t, in_=cv[0])
            cache_tiles[0][1] = t

        def emit_prefetch_late():
            t = singles.tile([128, CACHE], BF16, tag="ck1", name="ck1")
            nc.scalar.dma_start(out=t, in_=ckt[1])
            cache_tiles[1][0] = t
            t = singles.tile([128, NSC, HD], BF16, tag="cv1", name="cv1")
            nc.scalar.dma_start(out=t, in_=cv[1])
            cache_tiles[1][1] = t
            nc.scalar.dma_start(out=wo_sb, in_=wo)

        # ---- phase 1: projections -------------------------------------
        def proj_units(cs):
            """Generator: two units per (chunk, jc) = 2x8 matmuls + evac."""
            for c in cs:
                yield from _proj_chunk(c)

        def _proj_chunk(c):
            sl = slice(c * 512, c * 512 + 512)
            if c == 0:
                # x chunk 0 streams in per-DMA quarter tiles on sync while
                # wqkv quarters ride the scalar queue: tile-granular dep
                # tracking means a shared tile would make the first matmul
                # wait for every quarter
                xq = [xtp.tile([128, 4, 512], BF16, tag=f"xq{q}",
                               name=f"xq{q}") for q in range(4)]
                nc.sync.dma_start(out=xq[0], in_=xt4[0, :, 0:4, :])
                emit_prefetch()
                for q in range(1, 4):
                    nc.sync.dma_start(
                        out=xq[q], in_=xt4[0, :, 4 * q : 4 * q + 4, :]
                    )
            else:
                # chunks 1-3 in half tiles, one per hwdge queue
                xh = [xtp.tile([128, 8, 512], BF16, tag=f"xh{hq}",
                               name=f"xh{c}_{hq}") for hq in range(2)]
                nc.sync.dma_start(out=xh[0], in_=xt4[c, :, 0:8, :])
                nc.scalar.dma_start(out=xh[1], in_=xt4[c, :, 8:16, :])
            pps = {}
            if c == 0:
                # first chunk: iterate weight quarters outermost so matmuls
                # start as soon as the first wqkv quarter + x quarter land
                for jc in (3, 2, 0, 1):
                    pps[jc] = psS.tile([128, 512], F32, tag="ps",
                                       name=f"pp{c}_{jc}")
                for kq in range(4):
                    for jc in (3, 2, 0, 1):
                        for kk in range(kq * 4, kq * 4 + 4):
                            nc.tensor.matmul(
                                pps[jc],
                                wqkv_q[kk // 4][:, kk % 4,
                                                jc * 128 : jc * 128 + 128],
                                xq[kk // 4][:, kk % 4, :],
                                start=(kk == 0),
                                stop=(kk == ND - 1),
                            )
            for jc in (3, 2, 0, 1):  # v and k first
                if c == 0:
                    pp = pps[jc]
                else:
                    pp = psS.tile([128, 512], F32, tag="ps", name=f"pp{c}_{jc}")
                    for kk in range(ND):
                        nc.tensor.matmul(
                            pp,
                            wqkv_q[kk // 4][:, kk % 4,
                                            jc * 128 : jc * 128 + 128],
                            xh[kk // 8][:, kk % 8, :],
                            start=(kk == 0),
                            stop=(kk == ND - 1),
                        )
                        if kk == 7:
                            yield  # mid-chain filler granularity
                if jc == 3:
                    nc.vector.tensor_copy(vstage[:, sl], pp)
                else:
                    r = jc if jc < 2 else 2
                    nc.vector.tensor_copy(qkvt[:, r, sl], pp)
                    sq = wkp.tile([128, 512], BF16, tag="sq", name=f"sq{c}_{jc}")
                    nc.vector.tensor_mul(sq, qkvt[:, r, sl], qkvt[:, r, sl])
                    ssq = psS.tile([128, 512], F32, tag="ps", name=f"ssq{c}_{jc}")
                    p = 32 * r
                    nc.tensor.matmul(
                        ssq[p : p + 1, :], onesb, sq, start=True, stop=True
                    )
                    # raw sum-of-squares; sqrt happens 128-lane in column
                    # form inside finish_half so projection units emit no
                    # ACT work (keeps the Exp table loaded during
                    # interleaved attention)
                    nc.vector.tensor_copy(
                        sqr3[p : p + 1, sl], ssq[p : p + 1, :]
                    )
                yield

        def finish_half(half):
            """Reciprocal of the three rms rows (q0, q1, k): split each row
            to 8 partitions with a shape-preserving SBUF-SBUF DMA, PE-
            transpose into columns, 128-lane DVE reciprocal, transpose
            back, re-flatten; then rank-1 normalize of qkvt columns."""
            row_sl = slice(half * 1024, half * 1024 + 1024)
            st8 = colp.tile([8, 3, 128], F32R, tag="st8", name=f"st8_{half}")
            for r in range(3):
                nc.gpsimd.dma_start(
                    out=st8[:, r, :], in_=sqr3[32 * r : 32 * r + 1, row_sl]
                )
            tpc = psS.tile([128, 24], F32R, tag="ps", name=f"tpc{half}")
            for r in range(3):
                nc.tensor.transpose(
                    tpc[:, r * 8 : r * 8 + 8], st8[:, r, :], identr[0:8, 0:8]
                )
            # cols hold raw ssq: rsqrt = reciprocal(sqrt(ssq*scale + bias))
            # (q rows fold the 1/sqrt(HD) score scale into the bias form)
            sq_c = colp.tile([128, 24], F32R, tag="sqc", name=f"sqc{half}")
            for r in range(3):
                scale = 1.0 if r < 2 else 1.0 / HD
                bias = bias_q if r < 2 else bias_k
                nc.scalar.activation(
                    sq_c[:, r * 8 : r * 8 + 8], tpc[:, r * 8 : r * 8 + 8],
                    AF.Sqrt, bias=bias[:, :], scale=scale,
                )
            colsb = colp.tile([128, 24], F32R, tag="rcol", name=f"rcol{half}")
            nc.vector.reciprocal(colsb, sq_c)
            tpr = psS.tile([8, 384], F32R, tag="ps", name=f"tpr{half}")
            for r in range(3):
                nc.tensor.transpose(
                    tpr[:, r * 128 : r * 128 + 128],
                    colsb[:, r * 8 : r * 8 + 8],
                    identr,
                )
            st8b = colp.tile([8, 3, 128], F32R, tag="st8b", name=f"st8b{half}")
            for r in range(3):
                nc.vector.tensor_copy(
                    st8b[:, r, :], tpr[:, r * 128 : r * 128 + 128]
                )
            for r in range(3):
                nc.gpsimd.dma_start(
                    out=rr3[32 * r : 32 * r + 1, row_sl], in_=st8b[:, r, :]
                )
            for r in range(3):
                p = 32 * r
                for lc in range(2):
                    sl = slice(half * 1024 + lc * 512, half * 1024 + lc * 512 + 512)
                    bc = psS.tile([128, 512], F32, tag="ps",
                                  name=f"nbc{half}_{r}_{lc}")
                    nc.tensor.matmul(
                        bc, ones128[p : p + 1, :], rr3[p : p + 1, sl],
                        start=True, stop=True,
                    )
                    nc.vector.tensor_mul(qkvt[:, r, sl], qkvt[:, r, sl], bc)

        def emit_gates():
            # gates in column form: [l-part, chunk, head]
            gps = psS.tile([128, 16, QH], F32, tag="ps", name="gps")
            for cc in range(16):
                nc.tensor.matmul(
                    gps[:, cc, :],
                    xg_sb[:, cc * 128 : cc * 128 + 128],
                    wg_sb,
                    start=True,
                    stop=True,
                )
            nc.scalar.activation(gcol, gps, AF.Exp, scale=-1.0)
            nc.vector.tensor_scalar_add(gcol, gcol, 1.0)
            nc.vector.reciprocal(gcol, gcol)

        # ---- phase 2+3: attention and out-projection ------------------
        def prep_v(b, i0=0, i1=NSC):
            """Transpose new v [d,l] -> [s,d] bf16 for this batch."""
            if b not in vnew:
                vnew[b] = singles.tile([128, NSC, HD], BF16, tag=f"vn{b}",
                                       name=f"vn{b}")
            vn = vnew[b]
            for i in range(i0, i1):
                tp = psS.tile([128, 128], BF16, tag="ps", name=f"tp{b}_{i}")
                nc.tensor.transpose(
                    tp,
                    vstage[:, b * L + i * 128 : b * L + i * 128 + 128],
                    identb,
                )
                nc.vector.tensor_copy(vn[:, i, :], tp)

        def attn(b, h, filler=None, pending=None):
            """Attention for (batch b, local head h), pipelined 2 s-chunks
            ahead.  filler() emits one unit of foreign PE work per s-chunk
            from sc 5 on; pending is the previous block's deferred factor
            chain, emitted at s-chunk 4.  The softmax denominator is
            accumulated on DVE (bf16 running sum of the exp tiles) and
            partition-reduced with a single tiny matmul per l-half at the
            end, instead of 32 full-width st128x1 PE matmuls."""
            boff = b * L
            ck_sb, cv_sb = cache_tiles[b]
            ot = [psA.tile([128, 512], F32, tag="pa", name=f"ot{b}{h}{i}")
                  for i in range(2)]
            den = psA.tile([33, 512], F32, tag="pa", name=f"dn{b}{h}")
            acc = [accp.tile([128, 512], BF16, tag=f"ac{lh}",
                             name=f"ac{b}{h}{lh}") for lh in range(2)]
            exs = {}
            vxs = {}
            for sc in range(NS + 4):
                if sc < NS:
                    if sc < NSC:
                        kT = ck_sb[:, sc * 128 : sc * 128 + 128]
                        vxs[sc] = cv_sb[:, sc, :]
                    else:
                        j = boff + (sc - NSC) * 128
                        kT = qkvt[:, 2, j : j + 128]
                        vxs[sc] = vnew[b][:, sc - NSC, :]
                    for lh in range(2):
                        sp = psS.tile([128, 512], F32, tag="ps",
                                      name=f"sp{b}_{h}_{sc}_{lh}")
                        nc.tensor.matmul(
                            sp, kT,
                            qkvt[:, h, boff + lh * 512 : boff + lh * 512 + 512],
                            start=True, stop=True,
                        )
                        ex = exp_p.tile([128, 512], BF16, tag="ex",
                                        name=f"ex{b}_{h}_{sc}_{lh}")
                        nc.scalar.activation(ex, sp, AF.Exp)
                        exs[(sc, lh)] = ex
                        if sc == 1:
                            nc.vector.tensor_add(
                                acc[lh], exs[(0, lh)], ex
                            )
                        elif sc > 1:
                            nc.vector.tensor_add(acc[lh], acc[lh], ex)
                if sc >= 4:
                    sc2 = sc - 4
                    vx = vxs.pop(sc2)
                    for lh in range(2):
                        ex2 = exs.pop((sc2, lh))
                        nc.tensor.matmul(
                            ot[lh], vx, ex2,
                            start=(sc2 == 0), stop=(sc2 == NS - 1),
                        )
                    if sc == 5 and pending is not None:
                        pending()
                    if filler is not None and sc >= (6 if pending else 2):
                        filler()
            for lh in range(2):
                nc.tensor.matmul(
                    den[32 * lh : 32 * lh + 1, :], onesb, acc[lh],
                    start=True, stop=True,
                )
            # raw-evacuate attention out so the PSUM accumulators recycle
            for lh in range(2):
                nc.vector.tensor_copy(otg[:, b, h, lh, :], ot[lh])
            dnr = dnrp.tile([33, 512], F32R, tag="dnr", name=f"dnr{b}{h}")
            # ACT evacuates the den rows (it is idle at block boundaries
            # while DVE drains the otg copies), so the psA bank frees for
            # the next block's accumulators sooner
            for lh in range(2):
                nc.scalar.copy(
                    dnr[32 * lh : 32 * lh + 1, :],
                    den[32 * lh : 32 * lh + 1, :],
                )
            st4 = colp.tile([4, 2, 128], F32R, tag="st4", name=f"st4{b}{h}")
            for lh in range(2):
                nc.gpsimd.dma_start(
                    out=st4[:, lh, :], in_=dnr[32 * lh : 32 * lh + 1, :]
                )

            def finisher():
                # den rows -> columns (PE transpose), reciprocal, combine
                # with gates, back to rows, broadcast onto otg
                tpd = psS.tile([128, 8], F32R, tag="ps", name=f"tpd{b}{h}")
                for lh in range(2):
                    nc.tensor.transpose(
                        tpd[:, lh * 4 : lh * 4 + 4], st4[:, lh, :],
                        identr[0:4, 0:4],
                    )
                dcol = colp.tile([128, 8], F32R, tag="dcol", name=f"dc{b}{h}")
                nc.vector.reciprocal(dcol, tpd)
                fcol = colp.tile([128, 8], F32R, tag="fcol", name=f"fc{b}{h}")
                nc.vector.tensor_mul(
                    fcol, dcol, gcol[:, 8 * b : 8 * b + 8, h]
                )
                tpf = psS.tile([4, 256], F32R, tag="ps", name=f"tpf{b}{h}")
                st4b = colp.tile([4, 2, 128], F32R, tag="st4b",
                                 name=f"st4b{b}{h}")
                for lh in range(2):
                    nc.tensor.transpose(
                        tpf[:, lh * 128 : lh * 128 + 128],
                        fcol[:, lh * 4 : lh * 4 + 4],
                        identr,
                    )
                    nc.vector.tensor_copy(
                        st4b[:, lh, :], tpf[:, lh * 128 : lh * 128 + 128]
                    )
                    nc.gpsimd.dma_start(
                        out=f2[b][32 * h : 32 * h + 1,
                                  lh * 512 : lh * 512 + 512],
                        in_=st4b[:, lh, :],
                    )
                for lh in range(2):
                    bc = psS.tile([128, 512], F32, tag="ps",
                                  name=f"fbc{b}_{h}_{lh}")
                    nc.tensor.matmul(
                        bc, ones128[32 * h : 32 * h + 1, :],
                        f2[b][32 * h : 32 * h + 1, lh * 512 : lh * 512 + 512],
                        start=True, stop=True,
                    )
                    nc.vector.tensor_mul(
                        otg[:, b, h, lh, :], otg[:, b, h, lh, :], bc
                    )
            return finisher

        def ph3_units(b):
            """Generator of phase-3 units for batch b: each unit is one
            (lc2, li, mc-pair) -> two double-head [128,512] matmuls +
            evacuation, or a row DMA."""
            n = 0
            for lh in range(2):
                for li in range(4):
                    ysb = ysbp.tile([128, D], BF16, tag="ysb",
                                    name=f"ysb{b}_{lh}_{li}")
                    for mcp in range(2):
                        yps = []
                        for i in range(2):
                            mc = mcp * 2 + i
                            yp = psS.tile([128, 512], F32, tag="ps",
                                          name=f"yp{b}_{lh}_{li}_{mc}")
                            for hh in range(QH):
                                nc.tensor.matmul(
                                    yp,
                                    otg[:, b, hh, lh, li * 128 : li * 128 + 128],
                                    wo_sb[:, hh, mc * 512 : mc * 512 + 512],
                                    start=(hh == 0),
                                    stop=(hh == QH - 1),
                                )
                            yps.append((mc, yp))
                        for mc, yp in yps:
                            n += 1
                            if b == 1 and n % 2 == 0:
                                nc.scalar.copy(
                                    ysb[:, mc * 512 : mc * 512 + 512], yp
                                )
                            else:
                                nc.vector.tensor_copy(
                                    ysb[:, mc * 512 : mc * 512 + 512], yp
                                )
                        yield
                    row0 = b * L + lh * 512 + li * 128
                    if b == 0:
                        nc.sync.dma_start(out=y[row0 : row0 + 128, :], in_=ysb)
                    else:
                        # tail rows drain as per-queue halves
                        nc.sync.dma_start(
                            out=y[row0 : row0 + 128, 0:1024], in_=ysb[:, 0:1024]
                        )
                        nc.scalar.dma_start(
                            out=y[row0 : row0 + 128, 1024:2048],
                            in_=ysb[:, 1024:2048],
                        )
                    yield

        def drain(gen):
            if gen is not None:
                for _ in gen:
                    pass

        # ---- emission sequence ----------------------------------------
        onesb = singles.tile([128, 1], BF16)
        nc.vector.tensor_copy(onesb, ones128[:, 0:1])
        drain(proj_units(range(0, 1)))
        # gates and the first four v transposes need only chunk-0 data:
        # they fill the PE while the chunk-1 x tile streams in
        emit_gates()
        prep_v(0, 0, 4)
        drain(proj_units(range(1, 2)))
        emit_caches0()
        emit_prefetch_late()
        def late_units():
            finish_half(1)
            yield
            prep_v(1)
            yield

        import itertools
        gp = itertools.chain(proj_units(range(2, 4)), late_units())
        # remaining v transposes + three chunk-4 projection units first:
        # their PE work hides finish_half's bounce-chain latency
        prep_v(0, 4, NSC)
        for _ in range(3):
            next(gp, None)
        finish_half(0)
        # split the remaining projection units between the two batch-0
        # attention blocks so neither runs filler-dry
        gp_left = {"n": 8}
        def gp_limited():
            if gp_left["n"] > 0:
                gp_left["n"] -= 1
                return next(gp, None)
        fin = attn(0, 0, filler=gp_limited)
        fin = attn(0, 1, filler=lambda: next(gp, None), pending=fin)
        drain(gp)
        g0 = ph3_units(0)
        fin = attn(1, 0, filler=lambda: next(g0, None), pending=fin)
        # reserve a few phase-3 units so the last finisher's factor-chain
        # latency is hidden under leftover PE work
        g0_left = {"n": 6}
        def g0_limited():
            if g0_left["n"] > 0:
                g0_left["n"] -= 1
                return next(g0, None)
        fin = attn(1, 1, filler=g0_limited, pending=fin)
        fin()
        drain(g0)
        drain(ph3_units(1))

    nc.compile()
    return nc


def _get_nc():
    global _CACHED_NC
    if _CACHED_NC is None:
        _CACHED_NC = _build_core_program()
    return _CACHED_NC


def make_in_maps(x, w_q, w_k, w_v, w_out, w_gate, cache_k, cache_v):
    xt = np.ascontiguousarray(x.reshape(BL, D).T)         # [D, BL] f32
    # [4, 128, 16, 512]: chunk, partition, k-chunk, col
    xt4 = np.ascontiguousarray(
        xt.reshape(ND, 128, 4, 512).transpose(2, 1, 0, 3)
    ).astype(BF)
    xg = xt[0:H, :].astype(BF)
    identr = np.eye(128, dtype=np.float32)
    ones128_np = np.ones((128, 128), dtype=np.float32)
    in_maps = []
    for c in range(NCORES):
        g = c // 2
        wq_c = w_q[c * JC : (c + 1) * JC]                      # [256, D]
        wk_c = w_k[g * HD : (g + 1) * HD]                      # [128, D]
        wv_c = w_v[g * HD : (g + 1) * HD]
        wqkv_c = np.concatenate([wq_c, wk_c, wv_c], axis=0).T  # [D, 512]
        wqkv4 = np.ascontiguousarray(
            wqkv_c.reshape(4, 4, 128, 512).transpose(0, 2, 1, 3)
        ).astype(BF)                                           # [4,128,4,512]
        wo_c = np.ascontiguousarray(
            w_out[:, c * JC : (c + 1) * JC].T.reshape(QH, 128, D).transpose(1, 0, 2)
        ).astype(BF)                                           # [128, 2, D] bf16
        wg_c = np.ascontiguousarray(w_gate[c * QH : (c + 1) * QH].T).astype(BF)
        ckt_c = np.ascontiguousarray(
            cache_k[:, g].transpose(0, 2, 1)
        ).astype(BF)                                           # [B, HD, CACHE]
        cv_c = np.ascontiguousarray(
            cache_v[:, g].reshape(B, NSC, 128, HD).transpose(0, 2, 1, 3)
        ).astype(BF)                                           # [B,128,NSC,HD]
        in_maps.append(
            {
                "xt4": xt4,
                "wqkv": wqkv4,
                "wo": wo_c,
                "wg": wg_c,
                "xg": xg,
                "ckt": ckt_c,
                "cv": cv_c,
                "identr": identr,
                "ones_in": ones128_np,
            }
        )
    return in_maps


def kernel(x, w_q, w_k, w_v, w_out, w_gate, cache_k, cache_v, _run_kwargs=None):
    in_maps = make_in_maps(x, w_q, w_k, w_v, w_out, w_gate, cache_k, cache_v)
    nc = _get_nc()
    res = run_bass_kernel_spmd(
        nc, in_maps, core_ids=list(range(NCORES)), **(_run_kwargs or {})
    )
    acc = np.zeros((BL, D), dtype=np.float64)
    for c in range(NCORES):
        acc += np.asarray(res.results[c]["y"], dtype=np.float32)
    out = acc.astype(np.float32).reshape(B, L, D)
    if _run_kwargs:
        kernel.last_results = res
    return out



# revision 42
# speedup vs baseline: 1.0433x; 1.0059x over previous
"""Gated GQA self-attention with KV cache, tensor-parallel over heads on 8
Trainium2 NeuronCores.

Reference computation (fp32):
    q = rms_norm((x @ w_q.T).reshape(B,L,H,HD))      # per-head rms over HD
    k = rms_norm((x @ w_k.T).reshape(B,L,HKV,HD))
    v = (x @ w_v.T).reshape(B,L,HKV,HD)
    k_t/v_t = concat(cache, new) over seq -> [B,HKV,S,HD]
    o = softmax(q @ k_t.T / sqrt(HD)) @ v_t          # full (non-causal)
    o *= sigmoid(x[..., :16] @ w_gate.T)             # per-head gate
    y = o.reshape(B,L,D) @ w_out.T

Sharding: core c owns q heads {2c, 2c+1} and kv group g=c//2.  Each core
computes its heads' attention plus the partial out-projection
y_c = o_c @ w_out[:, cols_c].T; the host sums the 8 partials.

v5 design (baseline was 349us):
  * bf16 data plane for x / wqkv / caches / q / k / exp(p); float32r
    out-projection.  Host pre-chunks every tensor so each DMA moves
    contiguous 8KB-per-partition rows.
  * everything is computed feature-on-partition; scores come out [s, l]
    so the softmax matrix feeds the p@v matmul untransposed.
  * softmax denominator is computed OFF the tensor engine: exp tiles are
    pair-summed on DVE (bf16) and cross-partition-reduced on the idle
    GPSIMD engine (axis=C), saving 65k PE rows and two PSUM banks.
  * exp runs 1024 wide (one ACT op per s-chunk) with the score pipeline
    emitted two chunks ahead, so neither ACT latency nor per-op overhead
    stalls the PE.
  * tiny row<->column factor transposes ride the PE via identity
    transposes + shape-preserving DMAs (a transposing DMA emits 4-byte
    descriptors and takes ~15us); all of them are deferred off the PE
    critical path and issued on the otherwise idle Pool queue.
  * gate/denominator factors are folded into the raw attention output
    before the out-projection, so phase 3 accumulates both heads in PSUM
    and evacuates with one copy; phase 3 of batch 0 interleaves into
    batch 1's attention emission to fill PE stall slots.
"""

from contextlib import ExitStack

import numpy as np
import ml_dtypes

import concourse.bass as bass
import concourse.tile as tile
from concourse import bacc, mybir
from concourse.bass_utils import run_bass_kernel_spmd

F32R = mybir.dt.float32r
F32 = mybir.dt.float32
BF16 = mybir.dt.bfloat16
AF = mybir.ActivationFunctionType
AX = mybir.AxisListType
OP = mybir.AluOpType
BF = ml_dtypes.bfloat16

B, L, D = 2, 1024, 2048
H, HKV, HD = 16, 4, 128
CACHE = 1024
BL = B * L                  # 2048
S = CACHE + L               # 2048
NCORES = 8
QH = H // NCORES            # 2 q heads per core
JC = QH * HD                # 256 out-proj contraction cols per core
EPS = 1e-6
ND = D // 128               # 16 contraction chunks
NSC = CACHE // 128          # 8 cached s chunks per batch
NS = S // 128               # 16 s chunks per batch

_CACHED_NC = None


def _build_core_program():
    nc = bacc.Bacc("TRN2", target_bir_lowering=False, debug=False)

    xt4 = nc.dram_tensor("xt4", [4, 128, ND, 512], BF16, kind="ExternalInput").ap()
    wqkv = nc.dram_tensor("wqkv", [4, 128, 4, 512], BF16, kind="ExternalInput").ap()
    wo = nc.dram_tensor("wo", [128, QH, D], BF16, kind="ExternalInput").ap()
    wg = nc.dram_tensor("wg", [H, QH], BF16, kind="ExternalInput").ap()
    xg = nc.dram_tensor("xg", [H, BL], BF16, kind="ExternalInput").ap()
    ckt = nc.dram_tensor("ckt", [B, 128, CACHE], BF16, kind="ExternalInput").ap()
    cv = nc.dram_tensor("cv", [B, 128, NSC, HD], BF16, kind="ExternalInput").ap()
    identr_in = nc.dram_tensor("identr", [128, 128], F32R, kind="ExternalInput").ap()
    ones_in = nc.dram_tensor("ones_in", [128, 128], F32R, kind="ExternalInput").ap()
    y = nc.dram_tensor("y", [BL, D], BF16, kind="ExternalOutput").ap()

    with tile.TileContext(nc) as tc, ExitStack() as ctx:
        singles = ctx.enter_context(tc.tile_pool(name="singles", bufs=1))
        xtp = ctx.enter_context(tc.tile_pool(name="xtp", bufs=2))
        exp_p = ctx.enter_context(tc.tile_pool(name="exp_p", bufs=10))
        accp = ctx.enter_context(tc.tile_pool(name="accp", bufs=4))
        wkp = ctx.enter_context(tc.tile_pool(name="wkp", bufs=4))
        colp = ctx.enter_context(tc.tile_pool(name="colp", bufs=2))
        dnrp = ctx.enter_context(tc.tile_pool(name="dnrp", bufs=2))
        ysbp = ctx.enter_context(tc.tile_pool(name="ysbp", bufs=4))

        # PSUM (8 banks): psS = 5 x [128,512] transient ring (scores,
        # projection accumulators, transposes, broadcasts, phase-3
        # accumulators), psA = 3 x [128,512] persistent-per-block (p@v
        # accumulators x2 + both denominator rows stacked at partitions
        # 0/32 of one bank via matmul tile_position).
        psS = ctx.enter_context(tc.tile_pool(name="psS", bufs=5, space="PSUM"))
        psA = ctx.enter_context(tc.tile_pool(name="psA", bufs=3, space="PSUM"))

        lowp = nc.allow_low_precision(reason="bf16/f32r data plane is intended")
        ctx.enter_context(lowp)

        identr = singles.tile([128, 128], F32R)
        nc.scalar.dma_start(out=identr, in_=identr_in)
        ones128 = singles.tile([128, 128], F32R)
        nc.scalar.dma_start(out=ones128, in_=ones_in)
        onesr = ones128[0:1, :]
        identb = singles.tile([128, 128], BF16)
        nc.vector.tensor_copy(identb, identr)
        bias_q = singles.tile([128, 1], F32)
        nc.vector.memset(bias_q, HD * EPS)
        bias_k = singles.tile([128, 1], F32)
        nc.vector.memset(bias_k, EPS)

        wqkv_q = [singles.tile([128, 4, 512], BF16, tag=f"wq{k}",
                               name=f"wq{k}") for k in range(4)]
        nc.scalar.dma_start(out=wqkv_q[0], in_=wqkv[0])
        wo_sb = singles.tile([128, QH, D], BF16)
        wg_sb = singles.tile([H, QH], BF16)
        xg_sb = singles.tile([H, BL], BF16)

        # persistent activations, feature-on-partition
        qkvt = singles.tile([128, 3, BL], BF16)       # q0, q1, k (normalized)
        vstage = singles.tile([128, BL], BF16)        # raw v, [d, l]
        otg = singles.tile([128, B, QH, 2, 512], BF16)  # raw attn out [d, l]
        gcol = singles.tile([128, 16, QH], F32)       # gates, l-on-partition
        # rms rows packed at partitions 0/32/64 of one tile (one 8KB SBUF
        # column instead of three), which are also legal matmul bases
        sqr3 = singles.tile([65, BL], F32R)
        rr3 = singles.tile([65, BL], F32R)
        f2 = {b: singles.tile([33, 1024], F32R, tag=f"fr{b}", name=f"fr{b}")
              for b in range(B)}

        cache_tiles = {}
        vnew = {}

        def emit_prefetch():
            for kq in range(1, 4):
                nc.scalar.dma_start(out=wqkv_q[kq], in_=wqkv[kq])
            nc.scalar.dma_start(out=xg_sb, in_=xg)
            nc.scalar.dma_start(out=wg_sb, in_=wg)

        def emit_caches0():
            # batch-0 caches ride the scalar queue after the x1 half so
            # chunk 1 is never cache-blocked
            for b in range(B):
                cache_tiles.setdefault(b, [None, None])
            t = singles.tile([128, CACHE], BF16, tag="ck0", name="ck0")
            nc.scalar.dma_start(out=t, in_=ckt[0])
            cache_tiles[0][0] = t
            t = singles.tile([128, NSC, HD], BF16, tag="cv0", name="cv0")
            nc.scalar.dma_start(out=t, in_=cv[0])
            cache_tiles[0][1] = t

        def emit_prefetch_late():
            t = singles.tile([128, CACHE], BF16, tag="ck1", name="ck1")
            nc.scalar.dma_start(out=t, in_=ckt[1])
            cache_tiles[1][0] = t
            t = singles.tile([128, NSC, HD], BF16, tag="cv1", name="cv1")
            nc.scalar.dma_start(out=t, in_=cv[1])
            cache_tiles[1][1] = t
            nc.scalar.dma_start(out=wo_sb, in_=wo)

        # ---- phase 1: projections -------------------------------------
        def proj_units(cs):
            """Generator: two units per (chunk, jc) = 2x8 matmuls + evac."""
            for c in cs:
                yield from _proj_chunk(c)

        def _proj_chunk(c):
            sl = slice(c * 512, c * 512 + 512)
            if c == 0:
                # x chunk 0 streams in per-DMA quarter tiles on sync while
                # wqkv quarters ride the scalar queue: tile-granular dep
                # tracking means a shared tile would make the first matmul
                # wait for every quarter
                xq = [xtp.tile([128, 4, 512], BF16, tag=f"xq{q}",
                               name=f"xq{q}") for q in range(4)]
                nc.sync.dma_start(out=xq[0], in_=xt4[0, :, 0:4, :])
                emit_prefetch()
                for q in range(1, 4):
                    nc.sync.dma_start(
                        out=xq[q], in_=xt4[0, :, 4 * q : 4 * q + 4, :]
                    )
            else:
                # chunks 1-3 in half tiles, one per hwdge queue
                xh = [xtp.tile([128, 8, 512], BF16, tag=f"xh{hq}",
                               name=f"xh{c}_{hq}") for hq in range(2)]
                nc.sync.dma_start(out=xh[0], in_=xt4[c, :, 0:8, :])
                nc.scalar.dma_start(out=xh[1], in_=xt4[c, :, 8:16, :])
            pps = {}
            if c <= 1:
                # standalone chunks: iterate weight quarters outermost with
                # all four accumulators live, so matmuls start as soon as
                # the first granules land and the evacuation DVE chains
                # (copy -> square -> ssq) hide under the later jc chains
                xs = (lambda kk: xq[kk // 4][:, kk % 4, :]) if c == 0 else (
                    lambda kk: xh[kk // 8][:, kk % 8, :])
                for jc in (3, 2, 0, 1):
                    pps[jc] = psS.tile([128, 512], F32, tag="ps",
                                       name=f"pp{c}_{jc}")
                for kq in range(4):
                    for jc in (3, 2, 0, 1):
                        for kk in range(kq * 4, kq * 4 + 4):
                            nc.tensor.matmul(
                                pps[jc],
                                wqkv_q[kk // 4][:, kk % 4,
                                                jc * 128 : jc * 128 + 128],
                                xs(kk),
                                start=(kk == 0),
                                stop=(kk == ND - 1),
                            )
            for jc in (3, 2, 0, 1):  # v and k first
                if c <= 1:
                    pp = pps[jc]
                else:
                    pp = psS.tile([128, 512], F32, tag="ps", name=f"pp{c}_{jc}")
                    for kk in range(ND):
                        nc.tensor.matmul(
                            pp,
                            wqkv_q[kk // 4][:, kk % 4,
                                            jc * 128 : jc * 128 + 128],
                            xh[kk // 8][:, kk % 8, :],
                            start=(kk == 0),
                            stop=(kk == ND - 1),
                        )
                        if kk == 7:
                            yield  # mid-chain filler granularity
                if jc == 3:
                    nc.vector.tensor_copy(vstage[:, sl], pp)
                else:
                    r = jc if jc < 2 else 2
                    nc.vector.tensor_copy(qkvt[:, r, sl], pp)
                    sq = wkp.tile([128, 512], BF16, tag="sq", name=f"sq{c}_{jc}")
                    nc.vector.tensor_mul(sq, qkvt[:, r, sl], qkvt[:, r, sl])
                    ssq = psS.tile([128, 512], F32, tag="ps", name=f"ssq{c}_{jc}")
                    p = 32 * r
                    nc.tensor.matmul(
                        ssq[p : p + 1, :], onesb, sq, start=True, stop=True
                    )
                    # raw sum-of-squares; sqrt happens 128-lane in column
                    # form inside finish_half so projection units emit no
                    # ACT work (keeps the Exp table loaded during
                    # interleaved attention)
                    nc.vector.tensor_copy(
                        sqr3[p : p + 1, sl], ssq[p : p + 1, :]
                    )
                yield

        def finish_half(half):
            """Reciprocal of the three rms rows (q0, q1, k): split each row
            to 8 partitions with a shape-preserving SBUF-SBUF DMA, PE-
            transpose into columns, 128-lane DVE reciprocal, transpose
            back, re-flatten; then rank-1 normalize of qkvt columns."""
            row_sl = slice(half * 1024, half * 1024 + 1024)
            st8 = colp.tile([8, 3, 128], F32R, tag="st8", name=f"st8_{half}")
            for r in range(3):
                nc.gpsimd.dma_start(
                    out=st8[:, r, :], in_=sqr3[32 * r : 32 * r + 1, row_sl]
                )
            tpc = psS.tile([128, 24], F32R, tag="ps", name=f"tpc{half}")
            for r in range(3):
                nc.tensor.transpose(
                    tpc[:, r * 8 : r * 8 + 8], st8[:, r, :], identr[0:8, 0:8]
                )
            # cols hold raw ssq: rsqrt = reciprocal(sqrt(ssq*scale + bias))
            # (q rows fold the 1/sqrt(HD) score scale into the bias form)
            sq_c = colp.tile([128, 24], F32R, tag="sqc", name=f"sqc{half}")
            for r in range(3):
                scale = 1.0 if r < 2 else 1.0 / HD
                bias = bias_q if r < 2 else bias_k
                nc.scalar.activation(
                    sq_c[:, r * 8 : r * 8 + 8], tpc[:, r * 8 : r * 8 + 8],
                    AF.Sqrt, bias=bias[:, :], scale=scale,
                )
            colsb = colp.tile([128, 24], F32R, tag="rcol", name=f"rcol{half}")
            nc.vector.reciprocal(colsb, sq_c)
            tpr = psS.tile([8, 384], F32R, tag="ps", name=f"tpr{half}")
            for r in range(3):
                nc.tensor.transpose(
                    tpr[:, r * 128 : r * 128 + 128],
                    colsb[:, r * 8 : r * 8 + 8],
                    identr,
                )
            st8b = colp.tile([8, 3, 128], F32R, tag="st8b", name=f"st8b{half}")
            for r in range(3):
                nc.vector.tensor_copy(
                    st8b[:, r, :], tpr[:, r * 128 : r * 128 + 128]
                )
            for r in range(3):
                nc.gpsimd.dma_start(
                    out=rr3[32 * r : 32 * r + 1, row_sl], in_=st8b[:, r, :]
                )
            for r in range(3):
                p = 32 * r
                for lc in range(2):
                    sl = slice(half * 1024 + lc * 512, half * 1024 + lc * 512 + 512)
                    bc = psS.tile([128, 512], F32, tag="ps",
                                  name=f"nbc{half}_{r}_{lc}")
                    nc.tensor.matmul(
                        bc, ones128[p : p + 1, :], rr3[p : p + 1, sl],
                        start=True, stop=True,
                    )
                    nc.vector.tensor_mul(qkvt[:, r, sl], qkvt[:, r, sl], bc)

        def emit_gates():
            # gates in column form: [l-part, chunk, head]
            gps = psS.tile([128, 16, QH], F32, tag="ps", name="gps")
            for cc in range(16):
                nc.tensor.matmul(
                    gps[:, cc, :],
                    xg_sb[:, cc * 128 : cc * 128 + 128],
                    wg_sb,
                    start=True,
                    stop=True,
                )
            nc.scalar.activation(gcol, gps, AF.Exp, scale=-1.0)
            nc.vector.tensor_scalar_add(gcol, gcol, 1.0)
            nc.vector.reciprocal(gcol, gcol)

        # ---- phase 2+3: attention and out-projection ------------------
        def prep_v(b, i0=0, i1=NSC):
            """Transpose new v [d,l] -> [s,d] bf16 for this batch."""
            if b not in vnew:
                vnew[b] = singles.tile([128, NSC, HD], BF16, tag=f"vn{b}",
                                       name=f"vn{b}")
            vn = vnew[b]
            for i in range(i0, i1):
                tp = psS.tile([128, 128], BF16, tag="ps", name=f"tp{b}_{i}")
                nc.tensor.transpose(
                    tp,
                    vstage[:, b * L + i * 128 : b * L + i * 128 + 128],
                    identb,
                )
                nc.vector.tensor_copy(vn[:, i, :], tp)

        def attn(b, h, filler=None, pending=None):
            """Attention for (batch b, local head h), pipelined 2 s-chunks
            ahead.  filler() emits one unit of foreign PE work per s-chunk
            from sc 5 on; pending is the previous block's deferred factor
            chain, emitted at s-chunk 4.  The softmax denominator is
            accumulated on DVE (bf16 running sum of the exp tiles) and
            partition-reduced with a single tiny matmul per l-half at the
            end, instead of 32 full-width st128x1 PE matmuls."""
            boff = b * L
            ck_sb, cv_sb = cache_tiles[b]
            ot = [psA.tile([128, 512], F32, tag="pa", name=f"ot{b}{h}{i}")
                  for i in range(2)]
            den = psA.tile([33, 512], F32, tag="pa", name=f"dn{b}{h}")
            acc = [accp.tile([128, 512], BF16, tag=f"ac{lh}",
                             name=f"ac{b}{h}{lh}") for lh in range(2)]
            exs = {}
            vxs = {}
            for sc in range(NS + 4):
                if sc < NS:
                    if sc < NSC:
                        kT = ck_sb[:, sc * 128 : sc * 128 + 128]
                        vxs[sc] = cv_sb[:, sc, :]
                    else:
                        j = boff + (sc - NSC) * 128
                        kT = qkvt[:, 2, j : j + 128]
                        vxs[sc] = vnew[b][:, sc - NSC, :]
                    for lh in range(2):
                        sp = psS.tile([128, 512], F32, tag="ps",
                                      name=f"sp{b}_{h}_{sc}_{lh}")
                        nc.tensor.matmul(
                            sp, kT,
                            qkvt[:, h, boff + lh * 512 : boff + lh * 512 + 512],
                            start=True, stop=True,
                        )
                        ex = exp_p.tile([128, 512], BF16, tag="ex",
                                        name=f"ex{b}_{h}_{sc}_{lh}")
                        nc.scalar.activation(ex, sp, AF.Exp)
                        exs[(sc, lh)] = ex
                        if sc == 1:
                            nc.vector.tensor_add(
                                acc[lh], exs[(0, lh)], ex
                            )
                        elif sc > 1:
                            nc.vector.tensor_add(acc[lh], acc[lh], ex)
                if sc >= 4:
                    sc2 = sc - 4
                    vx = vxs.pop(sc2)
                    for lh in range(2):
                        ex2 = exs.pop((sc2, lh))
                        nc.tensor.matmul(
                            ot[lh], vx, ex2,
                            start=(sc2 == 0), stop=(sc2 == NS - 1),
                        )
                    if sc == 5 and pending is not None:
                        pending()
                    if filler is not None and sc >= (6 if pending else 2):
                        filler()
            for lh in range(2):
                nc.tensor.matmul(
                    den[32 * lh : 32 * lh + 1, :], onesb, acc[lh],
                    start=True, stop=True,
                )
            # raw-evacuate attention out so the PSUM accumulators recycle
            for lh in range(2):
                nc.vector.tensor_copy(otg[:, b, h, lh, :], ot[lh])
            dnr = dnrp.tile([33, 512], F32R, tag="dnr", name=f"dnr{b}{h}")
            # ACT evacuates the den rows (it is idle at block boundaries
            # while DVE drains the otg copies), so the psA bank frees for
            # the next block's accumulators sooner
            for lh in range(2):
                nc.scalar.copy(
                    dnr[32 * lh : 32 * lh + 1, :],
                    den[32 * lh : 32 * lh + 1, :],
                )
            st4 = colp.tile([4, 2, 128], F32R, tag="st4", name=f"st4{b}{h}")
            for lh in range(2):
                nc.gpsimd.dma_start(
                    out=st4[:, lh, :], in_=dnr[32 * lh : 32 * lh + 1, :]
                )

            def finisher():
                # den rows -> columns (PE transpose), reciprocal, combine
                # with gates, back to rows, broadcast onto otg
                tpd = psS.tile([128, 8], F32R, tag="ps", name=f"tpd{b}{h}")
                for lh in range(2):
                    nc.tensor.transpose(
                        tpd[:, lh * 4 : lh * 4 + 4], st4[:, lh, :],
                        identr[0:4, 0:4],
                    )
                dcol = colp.tile([128, 8], F32R, tag="dcol", name=f"dc{b}{h}")
                nc.vector.reciprocal(dcol, tpd)
                fcol = colp.tile([128, 8], F32R, tag="fcol", name=f"fc{b}{h}")
                nc.vector.tensor_mul(
                    fcol, dcol, gcol[:, 8 * b : 8 * b + 8, h]
                )
                tpf = psS.tile([4, 256], F32R, tag="ps", name=f"tpf{b}{h}")
                st4b = colp.tile([4, 2, 128], F32R, tag="st4b",
                                 name=f"st4b{b}{h}")
                for lh in range(2):
                    nc.tensor.transpose(
                        tpf[:, lh * 128 : lh * 128 + 128],
                        fcol[:, lh * 4 : lh * 4 + 4],
                        identr,
                    )
                    nc.vector.tensor_copy(
                        st4b[:, lh, :], tpf[:, lh * 128 : lh * 128 + 128]
                    )
                    nc.gpsimd.dma_start(
                        out=f2[b][32 * h : 32 * h + 1,
                                  lh * 512 : lh * 512 + 512],
                        in_=st4b[:, lh, :],
                    )
                for lh in range(2):
                    bc = psS.tile([128, 512], F32, tag="ps",
                                  name=f"fbc{b}_{h}_{lh}")
                    nc.tensor.matmul(
                        bc, ones128[32 * h : 32 * h + 1, :],
                        f2[b][32 * h : 32 * h + 1, lh * 512 : lh * 512 + 512],
                        start=True, stop=True,
                    )
                    nc.vector.tensor_mul(
                        otg[:, b, h, lh, :], otg[:, b, h, lh, :], bc
                    )
            return finisher

        def ph3_units(b):
            """Generator of phase-3 units for batch b: each unit is one
            (lc2, li, mc-pair) -> two double-head [128,512] matmuls +
            evacuation, or a row DMA."""
            n = 0
            for lh in range(2):
                for li in range(4):
                    ysb = ysbp.tile([128, D], BF16, tag="ysb",
                                    name=f"ysb{b}_{lh}_{li}")
                    for mcp in range(2):
                        yps = []
                        for i in range(2):
                            mc = mcp * 2 + i
                            yp = psS.tile([128, 512], F32, tag="ps",
                                          name=f"yp{b}_{lh}_{li}_{mc}")
                            for hh in range(QH):
                                nc.tensor.matmul(
                                    yp,
                                    otg[:, b, hh, lh, li * 128 : li * 128 + 128],
                                    wo_sb[:, hh, mc * 512 : mc * 512 + 512],
                                    start=(hh == 0),
                                    stop=(hh == QH - 1),
                                )
                            yps.append((mc, yp))
                        for mc, yp in yps:
                            n += 1
                            if b == 1 and n % 2 == 0:
                                nc.scalar.copy(
                                    ysb[:, mc * 512 : mc * 512 + 512], yp
                                )
                            else:
                                nc.vector.tensor_copy(
                                    ysb[:, mc * 512 : mc * 512 + 512], yp
                                )
                        yield
                    row0 = b * L + lh * 512 + li * 128
                    if b == 0:
                        nc.sync.dma_start(out=y[row0 : row0 + 128, :], in_=ysb)
                    else:
                        # tail rows drain as per-queue halves
                        nc.sync.dma_start(
                            out=y[row0 : row0 + 128, 0:1024], in_=ysb[:, 0:1024]
                        )
                        nc.scalar.dma_start(
                            out=y[row0 : row0 + 128, 1024:2048],
                            in_=ysb[:, 1024:2048],
                        )
                    yield

        def drain(gen):
            if gen is not None:
                for _ in gen:
                    pass

        # ---- emission sequence ----------------------------------------
        onesb = singles.tile([128, 1], BF16)
        nc.vector.tensor_copy(onesb, ones128[:, 0:1])
        drain(proj_units(range(0, 1)))
        # gates and the first four v transposes need only chunk-0 data:
        # they fill the PE while the chunk-1 x tile streams in
        emit_gates()
        prep_v(0, 0, 4)
        drain(proj_units(range(1, 2)))
        emit_caches0()
        emit_prefetch_late()
        def late_units():
            finish_half(1)
            yield
            prep_v(1)
            yield

        import itertools
        gp = itertools.chain(proj_units(range(2, 4)), late_units())
        # remaining v transposes + three chunk-4 projection units first:
        # their PE work hides finish_half's bounce-chain latency
        prep_v(0, 4, NSC)
        for _ in range(3):
            next(gp, None)
        finish_half(0)
        # split the remaining projection units between the two batch-0
        # attention blocks so neither runs filler-dry
        gp_left = {"n": 8}
        def gp_limited():
            if gp_left["n"] > 0:
                gp_left["n"] -= 1
                return next(gp, None)
        fin = attn(0, 0, filler=gp_limited)
        fin = attn(0, 1, filler=lambda: next(gp, None), pending=fin)
        drain(gp)
        g0 = ph3_units(0)
        fin = attn(1, 0, filler=lambda: next(g0, None), pending=fin)
        # reserve a few phase-3 units so the last finisher's factor-chain
        # latency is hidden under leftover PE work
        g0_left = {"n": 6}
        def g0_limited():
            if g0_left["n"] > 0:
                g0_left["n"] -= 1
                return next(g0, None)
        fin = attn(1, 1, filler=g0_limited, pending=fin)
        fin()
        drain(g0)
        drain(ph3_units(1))

    nc.compile()
    return nc


def _get_nc():
    global _CACHED_NC
    if _CACHED_NC is None:
        _CACHED_NC = _build_core_program()
    return _CACHED_NC


def make_in_maps(x, w_q, w_k, w_v, w_out, w_gate, cache_k, cache_v):
    xt = np.ascontiguousarray(x.reshape(BL, D).T)         # [D, BL] f32
    # [4, 128, 16, 512]: chunk, partition, k-chunk, col
    xt4 = np.ascontiguousarray(
        xt.reshape(ND, 128, 4, 512).transpose(2, 1, 0, 3)
    ).astype(BF)
    xg = xt[0:H, :].astype(BF)
    identr = np.eye(128, dtype=np.float32)
    ones128_np = np.ones((128, 128), dtype=np.float32)
    in_maps = []
    for c in range(NCORES):
        g = c // 2
        wq_c = w_q[c * JC : (c + 1) * JC]                      # [256, D]
        wk_c = w_k[g * HD : (g + 1) * HD]                      # [128, D]
        wv_c = w_v[g * HD : (g + 1) * HD]
        wqkv_c = np.concatenate([wq_c, wk_c, wv_c], axis=0).T  # [D, 512]
        wqkv4 = np.ascontiguousarray(
            wqkv_c.reshape(4, 4, 128, 512).transpose(0, 2, 1, 3)
        ).astype(BF)                                           # [4,128,4,512]
        wo_c = np.ascontiguousarray(
            w_out[:, c * JC : (c + 1) * JC].T.reshape(QH, 128, D).transpose(1, 0, 2)
        ).astype(BF)                                           # [128, 2, D] bf16
        wg_c = np.ascontiguousarray(w_gate[c * QH : (c + 1) * QH].T).astype(BF)
        ckt_c = np.ascontiguousarray(
            cache_k[:, g].transpose(0, 2, 1)
        ).astype(BF)                                           # [B, HD, CACHE]
        cv_c = np.ascontiguousarray(
            cache_v[:, g].reshape(B, NSC, 128, HD).transpose(0, 2, 1, 3)
        ).astype(BF)                                           # [B,128,NSC,HD]
        in_maps.append(
            {
                "xt4": xt4,
                "wqkv": wqkv4,
                "wo": wo_c,
                "wg": wg_c,
                "xg": xg,
                "ckt": ckt_c,
                "cv": cv_c,
                "identr": identr,
                "ones_in": ones128_np,
            }
        )
    return in_maps


def kernel(x, w_q, w_k, w_v, w_out, w_gate, cache_k, cache_v, _run_kwargs=None):
    in_maps = make_in_maps(x, w_q, w_k, w_v, w_out, w_gate, cache_k, cache_v)
    nc = _get_nc()
    res = run_bass_kernel_spmd(
        nc, in_maps, core_ids=list(range(NCORES)), **(_run_kwargs or {})
    )
    acc = np.zeros((BL, D), dtype=np.float64)
    for c in range(NCORES):
        acc += np.asarray(res.results[c]["y"], dtype=np.float32)
    out = acc.astype(np.float32).reshape(B, L, D)
    if _run_kwargs:
        kernel.last_results = res
    return out



# revision 43
# speedup vs baseline: 1.0441x; 1.0007x over previous
"""Gated GQA self-attention with KV cache, tensor-parallel over heads on 8
Trainium2 NeuronCores.

Reference computation (fp32):
    q = rms_norm((x @ w_q.T).reshape(B,L,H,HD))      # per-head rms over HD
    k = rms_norm((x @ w_k.T).reshape(B,L,HKV,HD))
    v = (x @ w_v.T).reshape(B,L,HKV,HD)
    k_t/v_t = concat(cache, new) over seq -> [B,HKV,S,HD]
    o = softmax(q @ k_t.T / sqrt(HD)) @ v_t          # full (non-causal)
    o *= sigmoid(x[..., :16] @ w_gate.T)             # per-head gate
    y = o.reshape(B,L,D) @ w_out.T

Sharding: core c owns q heads {2c, 2c+1} and kv group g=c//2.  Each core
computes its heads' attention plus the partial out-projection
y_c = o_c @ w_out[:, cols_c].T; the host sums the 8 partials.

v16 design (349 -> 292 -> 253 -> 240 -> 239us):
  * bf16 data plane everywhere (x / wqkv / wo / caches / q / k / v /
    exp(p) / attention out); fp32 only inside PSUM accumulation.
  * all matmuls use 512-wide moving operands: steady-state PE cadence is
    215ns per matmul (2.4 GHz, LDWEIGHTS hidden under the previous
    matmul's shadow-buffer load).
  * softmax denominator: DVE running bf16 sum of the exp tiles plus one
    tiny st128x1 reduce matmul per l-half at block end (v5 spent ~59us
    of PE time on full-width denominator matmuls).
  * gates: sigmoid evaluated via the Exp table (1/(1+e^-z), DVE finish)
    so no Sigmoid table load interrupts the attention exp stream.
  * DMA: each hwdge queue sustains only ~125-190 GB/s, and dependency
    tracking is tile-granular (a consumer waits for every DMA into its
    tile).  So x streams in per-DMA granule tiles - chunk 0 as four
    kq-aligned quarters on sync while the wqkv quarters ride the scalar
    queue, later chunks as one half per queue - batch-0 caches are
    issued after the x1 half, and the tail y rows drain as per-queue
    halves.
  * standalone projection chunks 0-1 run weight-quarter-outermost with
    all four psum accumulators live, so the evacuation DVE chains
    (copy -> square -> ssq row) hide under the later jc matmul chains.
  * projection chunks 2-3, the batch-1 rms factors, and the batch-0
    out-projection interleave into attention as PE filler units, with
    consumption capped so no block runs filler-dry and a few units stay
    in reserve to hide the last finisher's factor-chain latency.
  * tiny row<->column factor transposes ride the PE via identity
    transposes + shape-preserving DMAs; the gate/denominator factors are
    folded into the raw attention output before the out-projection.
"""

from contextlib import ExitStack

import numpy as np
import ml_dtypes

import concourse.bass as bass
import concourse.tile as tile
from concourse import bacc, mybir
from concourse.bass_utils import run_bass_kernel_spmd

F32R = mybir.dt.float32r
F32 = mybir.dt.float32
BF16 = mybir.dt.bfloat16
AF = mybir.ActivationFunctionType
AX = mybir.AxisListType
OP = mybir.AluOpType
BF = ml_dtypes.bfloat16

B, L, D = 2, 1024, 2048
H, HKV, HD = 16, 4, 128
CACHE = 1024
BL = B * L                  # 2048
S = CACHE + L               # 2048
NCORES = 8
QH = H // NCORES            # 2 q heads per core
JC = QH * HD                # 256 out-proj contraction cols per core
EPS = 1e-6
ND = D // 128               # 16 contraction chunks
NSC = CACHE // 128          # 8 cached s chunks per batch
NS = S // 128               # 16 s chunks per batch

_CACHED_NC = None


def _build_core_program():
    nc = bacc.Bacc("TRN2", target_bir_lowering=False, debug=False)

    xt4 = nc.dram_tensor("xt4", [4, 128, ND, 512], BF16, kind="ExternalInput").ap()
    wqkv = nc.dram_tensor("wqkv", [4, 128, 4, 512], BF16, kind="ExternalInput").ap()
    wo = nc.dram_tensor("wo", [128, QH, D], BF16, kind="ExternalInput").ap()
    wg = nc.dram_tensor("wg", [H, QH], BF16, kind="ExternalInput").ap()
    xg = nc.dram_tensor("xg", [H, BL], BF16, kind="ExternalInput").ap()
    ckt = nc.dram_tensor("ckt", [B, 128, CACHE], BF16, kind="ExternalInput").ap()
    cv = nc.dram_tensor("cv", [B, 128, NSC, HD], BF16, kind="ExternalInput").ap()
    identr_in = nc.dram_tensor("identr", [128, 128], F32R, kind="ExternalInput").ap()
    ones_in = nc.dram_tensor("ones_in", [128, 128], F32R, kind="ExternalInput").ap()
    y = nc.dram_tensor("y", [BL, D], BF16, kind="ExternalOutput").ap()

    with tile.TileContext(nc) as tc, ExitStack() as ctx:
        singles = ctx.enter_context(tc.tile_pool(name="singles", bufs=1))
        xtp = ctx.enter_context(tc.tile_pool(name="xtp", bufs=2))
        exp_p = ctx.enter_context(tc.tile_pool(name="exp_p", bufs=10))
        accp = ctx.enter_context(tc.tile_pool(name="accp", bufs=4))
        wkp = ctx.enter_context(tc.tile_pool(name="wkp", bufs=4))
        colp = ctx.enter_context(tc.tile_pool(name="colp", bufs=2))
        dnrp = ctx.enter_context(tc.tile_pool(name="dnrp", bufs=2))
        ysbp = ctx.enter_context(tc.tile_pool(name="ysbp", bufs=4))

        # PSUM (8 banks): psS = 5 x [128,512] transient ring (scores,
        # projection accumulators, transposes, broadcasts, phase-3
        # accumulators), psA = 3 x [128,512] persistent-per-block (p@v
        # accumulators x2 + both denominator rows stacked at partitions
        # 0/32 of one bank via matmul tile_position).
        psS = ctx.enter_context(tc.tile_pool(name="psS", bufs=5, space="PSUM"))
        psA = ctx.enter_context(tc.tile_pool(name="psA", bufs=3, space="PSUM"))

        lowp = nc.allow_low_precision(reason="bf16/f32r data plane is intended")
        ctx.enter_context(lowp)

        identr = singles.tile([128, 128], F32R)
        nc.scalar.dma_start(out=identr, in_=identr_in)
        ones128 = singles.tile([128, 128], F32R)
        nc.scalar.dma_start(out=ones128, in_=ones_in)
        onesr = ones128[0:1, :]
        identb = singles.tile([128, 128], BF16)
        nc.vector.tensor_copy(identb, identr)
        bias_q = singles.tile([128, 1], F32)
        nc.vector.memset(bias_q, HD * EPS)
        bias_k = singles.tile([128, 1], F32)
        nc.vector.memset(bias_k, EPS)

        wqkv_q = [singles.tile([128, 4, 512], BF16, tag=f"wq{k}",
                               name=f"wq{k}") for k in range(4)]
        nc.scalar.dma_start(out=wqkv_q[0], in_=wqkv[0])
        wo_sb = singles.tile([128, QH, D], BF16)
        wg_sb = singles.tile([H, QH], BF16)
        xg_sb = singles.tile([H, BL], BF16)

        # persistent activations, feature-on-partition
        qkvt = singles.tile([128, 3, BL], BF16)       # q0, q1, k (normalized)
        vstage = singles.tile([128, BL], BF16)        # raw v, [d, l]
        otg = singles.tile([128, B, QH, 2, 512], BF16)  # raw attn out [d, l]
        gcol = singles.tile([128, 16, QH], F32)       # gates, l-on-partition
        # rms rows packed at partitions 0/32/64 of one tile (one 8KB SBUF
        # column instead of three), which are also legal matmul bases
        sqr3 = singles.tile([65, BL], F32R)
        rr3 = singles.tile([65, BL], F32R)
        f2 = {b: singles.tile([33, 1024], F32R, tag=f"fr{b}", name=f"fr{b}")
              for b in range(B)}

        cache_tiles = {}
        vnew = {}

        def emit_prefetch():
            for kq in range(1, 4):
                nc.scalar.dma_start(out=wqkv_q[kq], in_=wqkv[kq])
            nc.scalar.dma_start(out=xg_sb, in_=xg)
            nc.scalar.dma_start(out=wg_sb, in_=wg)

        def emit_caches0():
            # batch-0 caches ride the scalar queue after the x1 half so
            # chunk 1 is never cache-blocked
            for b in range(B):
                cache_tiles.setdefault(b, [None, None])
            t = singles.tile([128, CACHE], BF16, tag="ck0", name="ck0")
            nc.scalar.dma_start(out=t, in_=ckt[0])
            cache_tiles[0][0] = t
            t = singles.tile([128, NSC, HD], BF16, tag="cv0", name="cv0")
            nc.scalar.dma_start(out=t, in_=cv[0])
            cache_tiles[0][1] = t

        def emit_prefetch_late():
            t = singles.tile([128, CACHE], BF16, tag="ck1", name="ck1")
            nc.scalar.dma_start(out=t, in_=ckt[1])
            cache_tiles[1][0] = t
            t = singles.tile([128, NSC, HD], BF16, tag="cv1", name="cv1")
            nc.scalar.dma_start(out=t, in_=cv[1])
            cache_tiles[1][1] = t
            nc.scalar.dma_start(out=wo_sb, in_=wo)

        # ---- phase 1: projections -------------------------------------
        def proj_units(cs):
            """Generator: two units per (chunk, jc) = 2x8 matmuls + evac."""
            for c in cs:
                yield from _proj_chunk(c)

        def _proj_chunk(c):
            sl = slice(c * 512, c * 512 + 512)
            if c == 0:
                # x chunk 0 streams in per-DMA quarter tiles on sync while
                # wqkv quarters ride the scalar queue: tile-granular dep
                # tracking means a shared tile would make the first matmul
                # wait for every quarter
                xq = [xtp.tile([128, 4, 512], BF16, tag=f"xq{q}",
                               name=f"xq{q}") for q in range(4)]
                nc.sync.dma_start(out=xq[0], in_=xt4[0, :, 0:4, :])
                emit_prefetch()
                for q in range(1, 4):
                    nc.sync.dma_start(
                        out=xq[q], in_=xt4[0, :, 4 * q : 4 * q + 4, :]
                    )
            else:
                # chunks 1-3 in half tiles, one per hwdge queue
                xh = [xtp.tile([128, 8, 512], BF16, tag=f"xh{hq}",
                               name=f"xh{c}_{hq}") for hq in range(2)]
                nc.sync.dma_start(out=xh[0], in_=xt4[c, :, 0:8, :])
                nc.scalar.dma_start(out=xh[1], in_=xt4[c, :, 8:16, :])
            pps = {}
            if c <= 1:
                # standalone chunks: iterate weight quarters outermost with
                # all four accumulators live, so matmuls start as soon as
                # the first granules land and the evacuation DVE chains
                # (copy -> square -> ssq) hide under the later jc chains
                xs = (lambda kk: xq[kk // 4][:, kk % 4, :]) if c == 0 else (
                    lambda kk: xh[kk // 8][:, kk % 8, :])
                for jc in (3, 2, 0, 1):
                    pps[jc] = psS.tile([128, 512], F32, tag="ps",
                                       name=f"pp{c}_{jc}")
                for kq in range(4):
                    for jc in (3, 2, 0, 1):
                        for kk in range(kq * 4, kq * 4 + 4):
                            nc.tensor.matmul(
                                pps[jc],
                                wqkv_q[kk // 4][:, kk % 4,
                                                jc * 128 : jc * 128 + 128],
                                xs(kk),
                                start=(kk == 0),
                                stop=(kk == ND - 1),
                            )
            for jc in (3, 2, 0, 1):  # v and k first
                if c <= 1:
                    pp = pps[jc]
                else:
                    pp = psS.tile([128, 512], F32, tag="ps", name=f"pp{c}_{jc}")
                    for kk in range(ND):
                        nc.tensor.matmul(
                            pp,
                            wqkv_q[kk // 4][:, kk % 4,
                                            jc * 128 : jc * 128 + 128],
                            xh[kk // 8][:, kk % 8, :],
                            start=(kk == 0),
                            stop=(kk == ND - 1),
                        )
                        if kk == 7:
                            yield  # mid-chain filler granularity
                if jc == 3:
                    nc.vector.tensor_copy(vstage[:, sl], pp)
                else:
                    r = jc if jc < 2 else 2
                    nc.vector.tensor_copy(qkvt[:, r, sl], pp)
                    sq = wkp.tile([128, 512], BF16, tag="sq", name=f"sq{c}_{jc}")
                    nc.vector.tensor_mul(sq, qkvt[:, r, sl], qkvt[:, r, sl])
                    ssq = psS.tile([128, 512], F32, tag="ps", name=f"ssq{c}_{jc}")
                    p = 32 * r
                    nc.tensor.matmul(
                        ssq[p : p + 1, :], onesb, sq, start=True, stop=True
                    )
                    # raw sum-of-squares; sqrt happens 128-lane in column
                    # form inside finish_half so projection units emit no
                    # ACT work (keeps the Exp table loaded during
                    # interleaved attention)
                    nc.vector.tensor_copy(
                        sqr3[p : p + 1, sl], ssq[p : p + 1, :]
                    )
                yield

        def finish_half(half):
            """Reciprocal of the three rms rows (q0, q1, k): split each row
            to 8 partitions with a shape-preserving SBUF-SBUF DMA, PE-
            transpose into columns, 128-lane DVE reciprocal, transpose
            back, re-flatten; then rank-1 normalize of qkvt columns."""
            row_sl = slice(half * 1024, half * 1024 + 1024)
            st8 = colp.tile([8, 3, 128], F32R, tag="st8", name=f"st8_{half}")
            for r in range(3):
                nc.gpsimd.dma_start(
                    out=st8[:, r, :], in_=sqr3[32 * r : 32 * r + 1, row_sl]
                )
            tpc = psS.tile([128, 24], F32R, tag="ps", name=f"tpc{half}")
            for r in range(3):
                nc.tensor.transpose(
                    tpc[:, r * 8 : r * 8 + 8], st8[:, r, :], identr[0:8, 0:8]
                )
            # cols hold raw ssq: rsqrt = reciprocal(sqrt(ssq*scale + bias))
            # (q rows fold the 1/sqrt(HD) score scale into the bias form)
            sq_c = colp.tile([128, 24], F32R, tag="sqc", name=f"sqc{half}")
            for r in range(3):
                scale = 1.0 if r < 2 else 1.0 / HD
                bias = bias_q if r < 2 else bias_k
                nc.scalar.activation(
                    sq_c[:, r * 8 : r * 8 + 8], tpc[:, r * 8 : r * 8 + 8],
                    AF.Sqrt, bias=bias[:, :], scale=scale,
                )
            colsb = colp.tile([128, 24], F32R, tag="rcol", name=f"rcol{half}")
            nc.vector.reciprocal(colsb, sq_c)
            tpr = psS.tile([8, 384], F32R, tag="ps", name=f"tpr{half}")
            for r in range(3):
                nc.tensor.transpose(
                    tpr[:, r * 128 : r * 128 + 128],
                    colsb[:, r * 8 : r * 8 + 8],
                    identr,
                )
            st8b = colp.tile([8, 3, 128], F32R, tag="st8b", name=f"st8b{half}")
            for r in range(3):
                nc.vector.tensor_copy(
                    st8b[:, r, :], tpr[:, r * 128 : r * 128 + 128]
                )
            for r in range(3):
                nc.gpsimd.dma_start(
                    out=rr3[32 * r : 32 * r + 1, row_sl], in_=st8b[:, r, :]
                )
            for r in range(3):
                p = 32 * r
                for lc in range(2):
                    sl = slice(half * 1024 + lc * 512, half * 1024 + lc * 512 + 512)
                    bc = psS.tile([128, 512], F32, tag="ps",
                                  name=f"nbc{half}_{r}_{lc}")
                    nc.tensor.matmul(
                        bc, ones128[p : p + 1, :], rr3[p : p + 1, sl],
                        start=True, stop=True,
                    )
                    nc.vector.tensor_mul(qkvt[:, r, sl], qkvt[:, r, sl], bc)

        def emit_gates():
            # gates in column form: [l-part, chunk, head]
            gps = psS.tile([128, 16, QH], F32, tag="ps", name="gps")
            for cc in range(16):
                nc.tensor.matmul(
                    gps[:, cc, :],
                    xg_sb[:, cc * 128 : cc * 128 + 128],
                    wg_sb,
                    start=True,
                    stop=True,
                )
            nc.scalar.activation(gcol, gps, AF.Exp, scale=-1.0)
            nc.vector.tensor_scalar_add(gcol, gcol, 1.0)
            nc.vector.reciprocal(gcol, gcol)

        # ---- phase 2+3: attention and out-projection ------------------
        def prep_v(b, i0=0, i1=NSC):
            """Transpose new v [d,l] -> [s,d] bf16 for this batch."""
            if b not in vnew:
                vnew[b] = singles.tile([128, NSC, HD], BF16, tag=f"vn{b}",
                                       name=f"vn{b}")
            vn = vnew[b]
            for i in range(i0, i1):
                tp = psS.tile([128, 128], BF16, tag="ps", name=f"tp{b}_{i}")
                nc.tensor.transpose(
                    tp,
                    vstage[:, b * L + i * 128 : b * L + i * 128 + 128],
                    identb,
                )
                nc.vector.tensor_copy(vn[:, i, :], tp)

        def attn(b, h, filler=None, pending=None):
            """Attention for (batch b, local head h), pipelined 2 s-chunks
            ahead.  filler() emits one unit of foreign PE work per s-chunk
            from sc 5 on; pending is the previous block's deferred factor
            chain, emitted at s-chunk 4.  The softmax denominator is
            accumulated on DVE (bf16 running sum of the exp tiles) and
            partition-reduced with a single tiny matmul per l-half at the
            end, instead of 32 full-width st128x1 PE matmuls."""
            boff = b * L
            ck_sb, cv_sb = cache_tiles[b]
            ot = [psA.tile([128, 512], F32, tag="pa", name=f"ot{b}{h}{i}")
                  for i in range(2)]
            den = psA.tile([33, 512], F32, tag="pa", name=f"dn{b}{h}")
            acc = [accp.tile([128, 512], BF16, tag=f"ac{lh}",
                             name=f"ac{b}{h}{lh}") for lh in range(2)]
            exs = {}
            vxs = {}
            for sc in range(NS + 4):
                if sc < NS:
                    if sc < NSC:
                        kT = ck_sb[:, sc * 128 : sc * 128 + 128]
                        vxs[sc] = cv_sb[:, sc, :]
                    else:
                        j = boff + (sc - NSC) * 128
                        kT = qkvt[:, 2, j : j + 128]
                        vxs[sc] = vnew[b][:, sc - NSC, :]
                    for lh in range(2):
                        sp = psS.tile([128, 512], F32, tag="ps",
                                      name=f"sp{b}_{h}_{sc}_{lh}")
                        nc.tensor.matmul(
                            sp, kT,
                            qkvt[:, h, boff + lh * 512 : boff + lh * 512 + 512],
                            start=True, stop=True,
                        )
                        ex = exp_p.tile([128, 512], BF16, tag="ex",
                                        name=f"ex{b}_{h}_{sc}_{lh}")
                        nc.scalar.activation(ex, sp, AF.Exp)
                        exs[(sc, lh)] = ex
                        if sc == 1:
                            nc.vector.tensor_add(
                                acc[lh], exs[(0, lh)], ex
                            )
                        elif sc > 1:
                            nc.vector.tensor_add(acc[lh], acc[lh], ex)
                if sc >= 4:
                    sc2 = sc - 4
                    vx = vxs.pop(sc2)
                    for lh in range(2):
                        ex2 = exs.pop((sc2, lh))
                        nc.tensor.matmul(
                            ot[lh], vx, ex2,
                            start=(sc2 == 0), stop=(sc2 == NS - 1),
                        )
                    if sc == 5 and pending is not None:
                        pending()
                    if filler is not None and sc >= (6 if pending else 2):
                        filler()
            for lh in range(2):
                nc.tensor.matmul(
                    den[32 * lh : 32 * lh + 1, :], onesb, acc[lh],
                    start=True, stop=True,
                )
            # raw-evacuate attention out so the PSUM accumulators recycle
            for lh in range(2):
                nc.vector.tensor_copy(otg[:, b, h, lh, :], ot[lh])
            dnr = dnrp.tile([33, 512], F32R, tag="dnr", name=f"dnr{b}{h}")
            # ACT evacuates the den rows (it is idle at block boundaries
            # while DVE drains the otg copies), so the psA bank frees for
            # the next block's accumulators sooner
            for lh in range(2):
                nc.scalar.copy(
                    dnr[32 * lh : 32 * lh + 1, :],
                    den[32 * lh : 32 * lh + 1, :],
                )
            st4 = colp.tile([4, 2, 128], F32R, tag="st4", name=f"st4{b}{h}")
            for lh in range(2):
                nc.gpsimd.dma_start(
                    out=st4[:, lh, :], in_=dnr[32 * lh : 32 * lh + 1, :]
                )

            def finisher():
                # den rows -> columns (PE transpose), reciprocal, combine
                # with gates, back to rows, broadcast onto otg
                tpd = psS.tile([128, 8], F32R, tag="ps", name=f"tpd{b}{h}")
                for lh in range(2):
                    nc.tensor.transpose(
                        tpd[:, lh * 4 : lh * 4 + 4], st4[:, lh, :],
                        identr[0:4, 0:4],
                    )
                dcol = colp.tile([128, 8], F32R, tag="dcol", name=f"dc{b}{h}")
                nc.vector.reciprocal(dcol, tpd)
                fcol = colp.tile([128, 8], F32R, tag="fcol", name=f"fc{b}{h}")
                nc.vector.tensor_mul(
                    fcol, dcol, gcol[:, 8 * b : 8 * b + 8, h]
                )
                tpf = psS.tile([4, 256], F32R, tag="ps", name=f"tpf{b}{h}")
                st4b = colp.tile([4, 2, 128], F32R, tag="st4b",
                                 name=f"st4b{b}{h}")
                for lh in range(2):
                    nc.tensor.transpose(
                        tpf[:, lh * 128 : lh * 128 + 128],
                        fcol[:, lh * 4 : lh * 4 + 4],
                        identr,
                    )
                    nc.vector.tensor_copy(
                        st4b[:, lh, :], tpf[:, lh * 128 : lh * 128 + 128]
                    )
                    nc.gpsimd.dma_start(
                        out=f2[b][32 * h : 32 * h + 1,
                                  lh * 512 : lh * 512 + 512],
                        in_=st4b[:, lh, :],
                    )
                for lh in range(2):
                    bc = psS.tile([128, 512], F32, tag="ps",
                                  name=f"fbc{b}_{h}_{lh}")
                    nc.tensor.matmul(
                        bc, ones128[32 * h : 32 * h + 1, :],
                        f2[b][32 * h : 32 * h + 1, lh * 512 : lh * 512 + 512],
                        start=True, stop=True,
                    )
                    nc.vector.tensor_mul(
                        otg[:, b, h, lh, :], otg[:, b, h, lh, :], bc
                    )
            return finisher

        def ph3_units(b):
            """Generator of phase-3 units for batch b: each unit is one
            (lc2, li, mc-pair) -> two double-head [128,512] matmuls +
            evacuation, or a row DMA."""
            n = 0
            for lh in range(2):
                for li in range(4):
                    ysb = ysbp.tile([128, D], BF16, tag="ysb",
                                    name=f"ysb{b}_{lh}_{li}")
                    for mcp in range(2):
                        yps = []
                        for i in range(2):
                            mc = mcp * 2 + i
                            yp = psS.tile([128, 512], F32, tag="ps",
                                          name=f"yp{b}_{lh}_{li}_{mc}")
                            for hh in range(QH):
                                nc.tensor.matmul(
                                    yp,
                                    otg[:, b, hh, lh, li * 128 : li * 128 + 128],
                                    wo_sb[:, hh, mc * 512 : mc * 512 + 512],
                                    start=(hh == 0),
                                    stop=(hh == QH - 1),
                                )
                            yps.append((mc, yp))
                        for mc, yp in yps:
                            n += 1
                            if b == 1 and n % 2 == 0:
                                nc.scalar.copy(
                                    ysb[:, mc * 512 : mc * 512 + 512], yp
                                )
                            else:
                                nc.vector.tensor_copy(
                                    ysb[:, mc * 512 : mc * 512 + 512], yp
                                )
                        yield
                    row0 = b * L + lh * 512 + li * 128
                    if b == 0:
                        nc.sync.dma_start(out=y[row0 : row0 + 128, :], in_=ysb)
                    else:
                        # tail rows drain as per-queue halves
                        nc.sync.dma_start(
                            out=y[row0 : row0 + 128, 0:1024], in_=ysb[:, 0:1024]
                        )
                        nc.scalar.dma_start(
                            out=y[row0 : row0 + 128, 1024:2048],
                            in_=ysb[:, 1024:2048],
                        )
                    yield

        def drain(gen):
            if gen is not None:
                for _ in gen:
                    pass

        # ---- emission sequence ----------------------------------------
        onesb = singles.tile([128, 1], BF16)
        nc.vector.tensor_copy(onesb, ones128[:, 0:1])
        drain(proj_units(range(0, 1)))
        # gates and the first four v transposes need only chunk-0 data:
        # they fill the PE while the chunk-1 x tile streams in
        emit_gates()
        prep_v(0, 0, 4)
        drain(proj_units(range(1, 2)))
        emit_caches0()
        emit_prefetch_late()
        def late_units():
            finish_half(1)
            yield
            prep_v(1)
            yield

        import itertools
        gp = itertools.chain(proj_units(range(2, 4)), late_units())
        # remaining v transposes + three chunk-4 projection units first:
        # their PE work hides finish_half's bounce-chain latency
        prep_v(0, 4, NSC)
        for _ in range(3):
            next(gp, None)
        finish_half(0)
        # split the remaining projection units between the two batch-0
        # attention blocks so neither runs filler-dry
        gp_left = {"n": 8}
        def gp_limited():
            if gp_left["n"] > 0:
                gp_left["n"] -= 1
                return next(gp, None)
        fin = attn(0, 0, filler=gp_limited)
        fin = attn(0, 1, filler=lambda: next(gp, None), pending=fin)
        drain(gp)
        g0 = ph3_units(0)
        fin = attn(1, 0, filler=lambda: next(g0, None), pending=fin)
        # reserve a few phase-3 units so the last finisher's factor-chain
        # latency is hidden under leftover PE work
        g0_left = {"n": 6}
        def g0_limited():
            if g0_left["n"] > 0:
                g0_left["n"] -= 1
                return next(g0, None)
        fin = attn(1, 1, filler=g0_limited, pending=fin)
        fin()
        drain(g0)
        drain(ph3_units(1))

    nc.compile()
    return nc


def _get_nc():
    global _CACHED_NC
    if _CACHED_NC is None:
        _CACHED_NC = _build_core_program()
    return _CACHED_NC


def make_in_maps(x, w_q, w_k, w_v, w_out, w_gate, cache_k, cache_v):
    xt = np.ascontiguousarray(x.reshape(BL, D).T)         # [D, BL] f32
    # [4, 128, 16, 512]: chunk, partition, k-chunk, col
    xt4 = np.ascontiguousarray(
        xt.reshape(ND, 128, 4, 512).transpose(2, 1, 0, 3)
    ).astype(BF)
    xg = xt[0:H, :].astype(BF)
    identr = np.eye(128, dtype=np.float32)
    ones128_np = np.ones((128, 128), dtype=np.float32)
    in_maps = []
    for c in range(NCORES):
        g = c // 2
        wq_c = w_q[c * JC : (c + 1) * JC]                      # [256, D]
        wk_c = w_k[g * HD : (g + 1) * HD]                      # [128, D]
        wv_c = w_v[g * HD : (g + 1) * HD]
        wqkv_c = np.concatenate([wq_c, wk_c, wv_c], axis=0).T  # [D, 512]
        wqkv4 = np.ascontiguousarray(
            wqkv_c.reshape(4, 4, 128, 512).transpose(0, 2, 1, 3)
        ).astype(BF)                                           # [4,128,4,512]
        wo_c = np.ascontiguousarray(
            w_out[:, c * JC : (c + 1) * JC].T.reshape(QH, 128, D).transpose(1, 0, 2)
        ).astype(BF)                                           # [128, 2, D] bf16
        wg_c = np.ascontiguousarray(w_gate[c * QH : (c + 1) * QH].T).astype(BF)
        ckt_c = np.ascontiguousarray(
            cache_k[:, g].transpose(0, 2, 1)
        ).astype(BF)                                           # [B, HD, CACHE]
        cv_c = np.ascontiguousarray(
            cache_v[:, g].reshape(B, NSC, 128, HD).transpose(0, 2, 1, 3)
        ).astype(BF)                                           # [B,128,NSC,HD]
        in_maps.append(
            {
                "xt4": xt4,
                "wqkv": wqkv4,
                "wo": wo_c,
                "wg": wg_c,
                "xg": xg,
                "ckt": ckt_c,
                "cv": cv_c,
                "identr": identr,
                "ones_in": ones128_np,
            }
        )
    return in_maps


def kernel(x, w_q, w_k, w_v, w_out, w_gate, cache_k, cache_v, _run_kwargs=None):
    in_maps = make_in_maps(x, w_q, w_k, w_v, w_out, w_gate, cache_k, cache_v)
    nc = _get_nc()
    res = run_bass_kernel_spmd(
        nc, in_maps, core_ids=list(range(NCORES)), **(_run_kwargs or {})
    )
    acc = np.zeros((BL, D), dtype=np.float64)
    for c in range(NCORES):
        acc += np.asarray(res.results[c]["y"], dtype=np.float32)
    out = acc.astype(np.float32).reshape(B, L, D)
    if _run_kwargs:
        kernel.last_results = res
    return out

